# revision 34
# baseline (speedup 1.0000x reference)
"""GATr model Bass kernel for 8 TRN2 NeuronCores.

Sharding: core i handles batch i//2, sequence half i%2 (2048 of 4096 tokens),
with x pre-sliced per core on the host. Per layer: LN + QKV run on the own
half; the pair's other half arrives via an AllGather of the residual state
issued right after the previous layer's write and consumed late, so it hides
under the own-half attention chunks; K/V for the remote half are recomputed
locally from it. Attention is flash-style over 32 kv chunks (own chunks
first) with a no-max-subtraction softmax and a ones-column in V producing
denominators in the same PSUM accumulation.

Precision: this network amplifies intermediate noise ~1000x into the output
(the geometric-product gating squares values with ~50x-rms outliers, and the
output tolerance sits only ~5x above the fp32 reassociation floor), so every
tensor stays fp32. The S logits matmul alone runs as the f32r hi/lo 3-term
reconstruction (kh*qh + kh*ql + kl*qh at 1 cyc/row each), which is
hardware-fp32-faithful at 3/4 the fp32 matmul cost.

Geometric product: blade-major s-layout [token, blade-pos(16), (chunk,ch)(80)]
with even blades in positions 0-7 and odd in 8-15 (w1 columns / w2 rows are
host-permuted to make this free). On its support the sign table factors
exactly as S[i,k] = (-1)^<lam(i), k XOR i> with unit effective sign, so terms
sharing a walsh plane reuse one r~ = r * walsh tile (stride-0 broadcast of a
[128,112] constant); each l-blade term is then a single DVE multiply against
an XOR-shuffled view of r~ (bit-runs become grouped dims, flipped runs become
reversals), and the 16 term tiles are accumulated exactly in PSUM by identity
matmuls on the otherwise-idle tensor engine. Odd l-blades only touch odd out
blades, halving their work.
"""

import numpy as np

INV, VM, H, NBLK = 32, 1, 5, 3
B, N = 4, 4096
NOWN = N // 2          # tokens per core (own sequence half)
NF = H * 16            # 80 features
METRIC = (0.0, 1.0, 1.0, 1.0)
T_RAW = (3, 5, 9)      # e01, e02, e03 raw blade masks
GRADE_RAW = [bin(m).count("1") for m in range(16)]
SPOS = [b // 2 if b % 2 == 0 else 8 + b // 2 for b in range(16)]  # s-layout
ISPOS = [0] * 16
for _b in range(16):
    ISPOS[SPOS[_b]] = _b
# permutation for the s-layout feature order: new (pos, ch) <- old (ch, blade)
PERM80 = np.array([[ch * 16 + ISPOS[p] for ch in range(5)] for p in range(16)]
                  ).reshape(-1)


# ---------------------------------------------------------------- host math
def _blade_mul(a, b):
    swaps = sum(bin(a >> (i + 1)).count("1") for i in range(4) if (b >> i) & 1)
    coef = -1.0 if swaps % 2 else 1.0
    for i in range(4):
        if ((a & b) >> i) & 1:
            coef *= METRIC[i]
    return a ^ b, coef


def _sign_table():
    S = np.zeros((16, 16), np.float32)  # S[i,k] = sign of l-blade i -> out k
    for i in range(16):
        for k in range(16):
            m, c = _blade_mul(i, k ^ i)
            S[i, k] = c if m == k else 0.0
    return S


def _walsh_lambdas():
    S = _sign_table()
    lams, effs = [], []
    for i in range(16):
        ks = [k for k in range(16) if S[i, k] != 0]
        hit = None
        for lam in range(16):
            for eps in (1.0, -1.0):
                if all(S[i, k] == eps * (-1) ** bin(lam & k).count("1")
                       for k in ks):
                    hit = (lam, eps)
                    break
            if hit:
                break
        assert hit is not None
        lam, eps = hit
        eff = eps * (-1) ** bin(lam & i).count("1")
        assert eff == 1.0, (i, eff)
        lams.append(lam)
    return lams


LAM = _walsh_lambdas()
NLAM = sorted(set(l for l in LAM if l != 0))  # 7 distinct nonzero lambdas


def _expand_equi(w, n_in, n_out):
    W = np.zeros((n_in * 16, n_out * 16), dtype=np.float32)
    for c in range(16):
        W[c::16, c::16] = w[GRADE_RAW[c]].T
    return W


def _build_consts(w_in, w_out, wq, wk, wv, wo, w_mlp1, w_mlp2):
    consts = {}
    inner = np.array([0.0 if (m & 1) else 1.0 for m in range(16)], np.float32)
    mask80 = np.tile(inner, H)
    scale = 1.0 / np.sqrt(16.0 * H)

    wemb = np.zeros((36, NF), np.float32)
    for o in range(H):
        for i in range(1, INV):
            wemb[i, o * 16] = w_in[0, o, i]
        wemb[35, o * 16] = w_in[0, o, 0]
        for d, c in enumerate(T_RAW):
            wemb[32 + d, o * 16 + c] = w_in[2, o, 0]
    consts["wemb"] = np.ascontiguousarray(wemb[:35])
    consts["bemb"] = np.ascontiguousarray(wemb[35:36].T)  # [80, 1] bias

    wsel = np.zeros((NF, 35), np.float32)
    for j in range(INV):
        for i in range(H):
            wsel[i * 16, j] = w_out[0, j, i]
    for d, c in enumerate(T_RAW):
        for i in range(H):
            wsel[i * 16 + c, 32 + d] = w_out[2, 0, i]
    consts["wsel"] = wsel

    for l in range(NBLK):
        consts[f"wq{l}"] = _expand_equi(wq[l], H, H)
        consts[f"wk{l}"] = (_expand_equi(wk[l], H, H)
                            * (mask80[None, :] * scale))
        consts[f"wv{l}"] = _expand_equi(wv[l], H, H)
        consts[f"wo{l}"] = _expand_equi(wo[l], H, H)
        w1 = _expand_equi(w_mlp1[l], H, 2 * H)   # [80, 160]
        consts[f"w1a{l}"] = np.ascontiguousarray(w1[:, :NF][:, PERM80])
        consts[f"w1b{l}"] = np.ascontiguousarray(w1[:, NF:][:, PERM80])
        consts[f"w2{l}"] = np.ascontiguousarray(
            _expand_equi(w_mlp2[l], H, H)[PERM80, :])

    # walsh sign planes over s-layout positions, one 16-wide row per
    # nonzero lambda; broadcast over ck at use via a stride-0 last dim
    wl = np.zeros((len(NLAM), 16), np.float32)
    for vi, lam in enumerate(NLAM):
        for p in range(16):
            wl[vi, p] = (-1.0) ** bin(lam & ISPOS[p]).count("1")
    consts["walsh"] = np.ascontiguousarray(
        np.broadcast_to(wl.reshape(1, -1), (128, len(NLAM) * 16)))
    consts["lnmask"] = np.ascontiguousarray(mask80.reshape(NF, 1))
    return consts


# ---------------------------------------------------------------- bass build
_BUILT = {}
TAPS = []  # debug: intermediate tile names to dump as extra outputs


def _tap(nc, tc, sb, name, tile, shape, dtype):
    if name not in TAPS:
        return
    import concourse.mybir as mybir
    p = nc.declare_dram_parameter(f"tap_{name}", shape, dtype, isOutput=True)
    nc.sync.dma_start(p[:], tile)


def _split_excess_waits(nc, max_waits=1):
    """walrus setupSyncWait rejects >1 sem wait on one instruction; move
    excess waits onto same-engine carrier nops placed just before."""
    import bass_rust

    for bb in nc.main_func.blocks:
        il = bb.instructions
        i = 0
        while i < len(il):
            ins = il[i]
            si = ins.sync_info
            if si is not None and si.on_wait and len(si.on_wait) > max_waits:
                waits = list(si.on_wait)
                si.on_wait = waits[:max_waits]
                excess = waits[max_waits:]
                carriers = []
                for j in range(0, len(excess), max_waits):
                    nop = nc.engines[ins.engine].nop(nofuse=True)
                    nop.ins.sync_info = bass_rust.SyncInfo(
                        on_wait=excess[j : j + max_waits], on_update=[]
                    )
                    carriers.append(nop.ins)
                for bb2 in nc.main_func.blocks:
                    il2 = bb2.instructions
                    for c in carriers:
                        for k in range(len(il2) - 1, -1, -1):
                            if il2[k].name == c.name:
                                il2.pop(k)
                i = next(k for k, x in enumerate(il) if x.name == ins.name)
                for c in reversed(carriers):
                    il.insert(i, c)
                i += len(carriers)
            i += 1


def _build_nc(reps=1):
    import concourse.bass as bass
    import concourse.mybir as mybir
    import concourse.tile as tile
    from concourse import bacc
    from concourse.bass import ds
    from concourse.masks import make_identity

    f32 = mybir.dt.float32
    f32r = mybir.dt.float32r
    f16 = mybir.dt.float16
    bf16 = mybir.dt.bfloat16
    AF = mybir.ActivationFunctionType
    ALU = mybir.AluOpType

    nc = bacc.Bacc("TRN2", target_bir_lowering=False, debug=False, num_devices=8)

    x_in = nc.declare_dram_parameter("x", [NOWN, 35], f32, isOutput=False)
    out_p = nc.declare_dram_parameter("out", [NOWN, 35], f32, isOutput=True)
    shapes = {"wemb": [35, NF], "bemb": [NF, 1], "wsel": [NF, 35],
              "walsh": [128, len(NLAM) * 16], "lnmask": [NF, 1]}
    dtypes = {"wemb": f32, "bemb": f32, "wsel": f32, "walsh": f32,
              "lnmask": f32}
    wnames = ["wemb", "bemb", "wsel", "walsh", "lnmask"]
    for l in range(NBLK):
        for nm in ("wq", "wk", "wv", "wo", "w1a", "w1b", "w2"):
            wnames.append(f"{nm}{l}")
            shapes[f"{nm}{l}"] = [NF, NF]
            dtypes[f"{nm}{l}"] = f32
    wp = {
        name: nc.declare_dram_parameter(name, shapes[name], dtypes[name],
                                        isOutput=False)
        for name in wnames
    }

    with tile.TileContext(nc) as tc:
        pid = nc.partition_id()
        roff = ((pid + 1) % 2) * NF   # remote half rows in the AllGather out

        cst_cm = tc.tile_pool(name="cst", bufs=1)
        cst = cst_cm.__enter__()
        idn = cst.tile([128, 128], f32, name="idn")
        make_identity(nc, idn[:])
        ones_f = cst.tile([1, NF], f32, name="ones_f")
        nc.vector.memset(ones_f[:], 1.0)
        eps_t = cst.tile([1, 1], f32, name="eps_t")
        nc.vector.memset(eps_t[:], 1e-6)
        wsb = {}
        for name in wnames:
            t = cst.tile(shapes[name], dtypes[name], name=f"sb_{name}")
            nc.sync.dma_start(t[:], wp[name][:])
            wsb[name] = t

        for _rep in range(reps):
            sb_cm = tc.tile_pool(name=f"sb_{_rep}", bufs=1)
            sb = sb_cm.__enter__()

            # ------------- embedding: own x -> h_own [80, 2048] fp32
            h_own = sb.tile([NF, NOWN], f32, name="h_emb", tag="h_own", bufs=1)
            with tc.tile_pool(name=f"pemb_{_rep}", bufs=2, space="PSUM") as pemb, \
                 tc.tile_pool(name=f"sbemb_{_rep}", bufs=3) as sbemb:
                for s in range(NOWN // 512):
                    xa = sbemb.tile([128, 4 * 35], f32, name="xa", bufs=2)
                    xav = xa[:].rearrange("p (j c) -> p j c", j=4, c=35)
                    nc.sync.dma_start(
                        xav, x_in[:].rearrange("(s j p) c -> s p j c",
                                               s=4, j=4, p=128)[s])
                    xT = sbemb.tile([35, 512], f32, name="xT", bufs=2)
                    for j in range(4):
                        pxt = pemb.tile([35, 128], f32, name="pxt", bufs=2)
                        nc.tensor.transpose(pxt[:], xav[:, j, :], idn[:])
                        nc.vector.tensor_copy(xT[0:35, j * 128:(j + 1) * 128],
                                              pxt[:])
                    pe = pemb.tile([NF, 512], f32, name="pe", bufs=2)
                    nc.tensor.matmul(pe[:], wsb["wemb"][:], xT[:],
                                     start=True, stop=True)
                    nc.vector.tensor_scalar_add(
                        h_own[:, s * 512:(s + 1) * 512], pe[:], wsb["bemb"][:])

            # ------------- pair exchange of the residual state (fp16)
            dcc_cm = tc.tile_pool(name=f"dcc_{_rep}", bufs=1, space="DRAM")
            dcc = dcc_cm.__enter__()

            def exchange(h_src, idx):
                ccin = dcc.tile([NF, NOWN], f32, name=f"ccin{idx}")
                ccout = dcc.tile([2 * NF, NOWN], f32, name=f"ccout{idx}")
                nc.sync.dma_start(ccin[:], h_src[:])
                nc.gpsimd.collective_compute(
                    "AllGather", mybir.AluOpType.bypass,
                    replica_groups=[[0, 1], [2, 3], [4, 5], [6, 7]],
                    ins=[ccin[:]], outs=[ccout[:]])
                h_r = sb.tile([NF, NOWN], f32, name=f"hr{idx}", tag="h_r",
                              bufs=2)
                nc.sync.dma_start(h_r[:], ccout[ds(roff, NF), :])
                return h_r

            h_r = exchange(h_own, 0)
            _tap(nc, tc, sb, "h0", h_own[:], [NF, NOWN], f32)
            _tap(nc, tc, sb, "hr0", h_r[:], [NF, NOWN], f16)

            # ------------- layer norm: src [80, n] -> dst (fp16), chunked
            def layer_norm(src, dst, n, name, pbufs=2):
                rdt = f32
                lhs_m = wsb["lnmask"]
                lhs_o = ones_f
                with tc.tile_pool(name=f"pln_{name}_{_rep}", bufs=1,
                                  space="PSUM") as pln, \
                     tc.tile_pool(name=f"sln_{name}_{_rep}", bufs=2) as sln:
                    for s in range(n // 512):
                        sl = slice(s * 512, (s + 1) * 512)
                        sq = sln.tile([NF, 512], rdt, name="sq", bufs=2)
                        nc.vector.tensor_tensor(sq[:], src[:, sl], src[:, sl],
                                                ALU.mult)
                        pip = pln.tile([1, 512], f32, name="pip",
                                       bufs=pbufs)
                        nc.tensor.matmul(pip[:], lhs_m[:], sq[:],
                                         start=True, stop=True)
                        sd = sln.tile([1, 512], f32, name="sd", bufs=2)
                        nc.scalar.activation(sd[:], pip[:], AF.Sqrt,
                                             bias=eps_t[:], scale=1.0 / float(NF))
                        rs = sln.tile([1, 512], rdt, name="rs", bufs=2)
                        nc.vector.reciprocal(rs[:], sd[:])
                        pbc = pln.tile([NF, 512], f32, name="pbc",
                                       bufs=pbufs)
                        nc.tensor.matmul(pbc[:], lhs_o[:], rs[:],
                                         start=True, stop=True)
                        nc.vector.tensor_tensor(dst[:, sl], src[:, sl], pbc[:],
                                                ALU.mult)

            # ------------- layers
            for l in range(NBLK):
                # LN + QKV: own half fully first (so the PE stream never
                # head-of-line blocks on the in-flight exchange), then remote
                hln = sb.tile([NF, N], f32, name=f"hln{l}", tag="hln", bufs=1)
                _tap(nc, tc, sb, f"hln{l}", hln[:], [NF, N], f32)
                qh = sb.tile([NF, NOWN], f32r, name=f"qh{l}", tag="qh", bufs=1)
                ql = sb.tile([NF, NOWN], f32r, name=f"ql{l}", tag="ql", bufs=1)
                kh = sb.tile([NF, N], f32r, name=f"kh{l}", tag="kh", bufs=1)
                kl = sb.tile([NF, N], f32r, name=f"kl{l}", tag="kl", bufs=1)
                vtok = sb.tile([128, 32 * 81], f32, name=f"v{l}", tag="v",
                               bufs=1)
                nc.vector.memset(
                    vtok[:].rearrange("p (c u) -> p c u", c=32, u=81)[:, :, 80:81],
                    1.0)
                with tc.tile_pool(name=f"pqkv_{l}_{_rep}", bufs=2,
                                  space="PSUM") as pqkv:
                    layer_norm(h_own, hln[:, 0:NOWN], NOWN, f"a{l}")
                    for s in range(NOWN // 512):
                        sl = slice(s * 512, (s + 1) * 512)
                        pq = pqkv.tile([NF, 512], f32, name="pqk", bufs=2)
                        nc.tensor.matmul(pq[:], wsb[f"wq{l}"][:], hln[:, sl],
                                         start=True, stop=True)
                        nc.vector.tensor_copy(qh[:, sl], pq[:])
                        nc.vector.tensor_tensor(ql[:, sl], pq[:],
                                                qh[:, sl].bitcast(f32),
                                                ALU.subtract)
                        pk = pqkv.tile([NF, 512], f32, name="pqk", bufs=2)
                        nc.tensor.matmul(pk[:], wsb[f"wk{l}"][:], hln[:, sl],
                                         start=True, stop=True)
                        nc.vector.tensor_copy(kh[:, sl], pk[:])
                        nc.vector.tensor_tensor(kl[:, sl], pk[:],
                                                kh[:, sl].bitcast(f32),
                                                ALU.subtract)
                    for c in range(16):
                        pv = pqkv.tile([128, NF], f32, name="pv", bufs=2)
                        nc.tensor.matmul(pv[:], hln[:, c * 128:(c + 1) * 128],
                                         wsb[f"wv{l}"][:],
                                         start=True, stop=True)
                        nc.vector.tensor_copy(vtok[:, c * 81:c * 81 + 80],
                                              pv[:])

                # attention: flash accumulation into psAV [81, 2048]; own kv
                # chunks run while the exchange lands, then remote LN/K/V are
                # produced in small psum pools coexisting with the S pipeline
                avsn = sb.tile([NF, NOWN], f32, name=f"avsn{l}", tag="avsn",
                               bufs=1)
                h_att = sb.tile([NF, NOWN], f32, name=f"hatt{l}", tag="h_att",
                                bufs=1)
                with tc.tile_pool(name=f"pAV_{l}_{_rep}", bufs=1,
                                  space="PSUM") as pAV:
                    psAV = pAV.tile([81, NOWN], f32, name="psAV")

                    def s_chunks(c0, c1):
                        with tc.tile_pool(name=f"pS_{l}_{c0}_{_rep}", bufs=2,
                                          space="PSUM") as pS, \
                             tc.tile_pool(name=f"sP_{l}_{c0}_{_rep}",
                                          bufs=3) as sP:
                            for c in range(c0, c1):
                                kchh = kh[:, c * 128:(c + 1) * 128]
                                kchl = kl[:, c * 128:(c + 1) * 128]
                                vch = vtok[:, c * 81:c * 81 + 81]
                                for ns in range(NOWN // 512):
                                    psS = pS.tile([128, 512], f32, name="psS",
                                                  bufs=2)
                                    qsl = slice(ns * 512, (ns + 1) * 512)
                                    nc.tensor.matmul(psS[:], kchh, qh[:, qsl],
                                                     start=True, stop=False)
                                    nc.tensor.matmul(psS[:], kchh, ql[:, qsl],
                                                     start=False, stop=False)
                                    nc.tensor.matmul(psS[:], kchl, qh[:, qsl],
                                                     start=False, stop=True)
                                    pT = sP.tile([128, 512], f32, name="pT",
                                                 bufs=3)
                                    nc.scalar.activation(pT[:], psS[:], AF.Exp)
                                    nc.tensor.matmul(
                                        psAV[:, qsl], vch, pT[:],
                                        start=(c == 0), stop=(c == 31))

                    s_chunks(0, 16)
                    # remote half: LN then K/V in their own small pools
                    layer_norm(h_r, hln[:, NOWN:N], NOWN, f"b{l}",
                               pbufs=1)
                    with tc.tile_pool(name=f"pqkr_{l}_{_rep}", bufs=1,
                                      space="PSUM") as pqkr:
                        for s in range(NOWN // 512):
                            sl = slice(NOWN + s * 512, NOWN + (s + 1) * 512)
                            pk = pqkr.tile([NF, 512], f32, name="pk", bufs=1)
                            nc.tensor.matmul(pk[:], wsb[f"wk{l}"][:],
                                             hln[:, sl], start=True, stop=True)
                            nc.scalar.copy(kh[:, sl], pk[:])
                            nc.vector.tensor_tensor(kl[:, sl], pk[:],
                                                    kh[:, sl].bitcast(f32),
                                                    ALU.subtract)
                        for c in range(16, 32):
                            pv = pqkr.tile([128, NF], f32, name="pv", bufs=1)
                            nc.tensor.matmul(pv[:],
                                             hln[:, c * 128:(c + 1) * 128],
                                             wsb[f"wv{l}"][:],
                                             start=True, stop=True)
                            nc.vector.tensor_copy(vtok[:, c * 81:c * 81 + 80],
                                                  pv[:])
                    s_chunks(16, 32)

                    # denominators + normalized attention output
                    avs = sb.tile([81, NOWN], f32, name=f"avs{l}", tag="avs",
                                  bufs=1)
                    nc.vector.tensor_copy(avs[:], psAV[:])
                    dnm = sb.tile([1, NOWN], f32, name=f"dnm{l}", tag="dnm",
                                  bufs=1)
                    nc.sync.dma_start(dnm[:], avs[80:81, :])
                    rd = dnm
                    nc.vector.reciprocal(rd[:], dnm[:])
                    with tc.tile_pool(name=f"ppost_{l}_{_rep}", bufs=2,
                                      space="PSUM") as ppost:
                        for s in range(NOWN // 512):
                            sl = slice(s * 512, (s + 1) * 512)
                            pbb = ppost.tile([NF, 512], f32, name="pbb", bufs=2)
                            nc.tensor.matmul(pbb[:], ones_f[:], rd[:, sl],
                                             start=True, stop=True)
                            nc.vector.tensor_tensor(avsn[:, sl], avs[0:NF, sl],
                                                    pbb[:], ALU.mult)
                with tc.tile_pool(name=f"pwo_{l}_{_rep}", bufs=2,
                                  space="PSUM") as pwo:
                    for s in range(NOWN // 512):
                        sl = slice(s * 512, (s + 1) * 512)
                        pw = pwo.tile([NF, 512], f32, name="pw", bufs=2)
                        nc.tensor.matmul(pw[:], wsb[f"wo{l}"][:], avsn[:, sl],
                                         start=True, stop=True)
                        nc.vector.tensor_tensor(h_att[:, sl], pw[:],
                                                h_own[:, sl], ALU.add)

                # ---------------- MLP on own half
                hln2 = sb.tile([NF, NOWN], f32, name=f"hln2{l}", tag="hln2",
                               bufs=1)
                layer_norm(h_att, hln2[:], NOWN, f"m{l}")

                # ltok/rtok layout per partition: pos(16) x chunk(16) x ch(5);
                # w1a/w1b columns are host-permuted to (pos, ch) order, so the
                # per-chunk [128, 80] matmul output scatters into the chunk
                # column of the (pos, chunk, ch) grid.
                ltok = sb.tile([128, 1280], f32, name=f"ltok{l}", tag="ltok",
                               bufs=1)
                rtok = sb.tile([128, 1280], f32, name=f"rtok{l}", tag="rtok",
                               bufs=1)
                l4 = ltok[:].rearrange("p (b t c) -> p b t c", b=16, t=16, c=5)
                r4 = rtok[:].rearrange("p (b t c) -> p b t c", b=16, t=16, c=5)
                with tc.tile_pool(name=f"plr_{l}_{_rep}", bufs=2,
                                  space="PSUM") as plr:
                    for g in range(4):
                        pl = plr.tile([128, 320], f32, name="pl", bufs=2)
                        pr = plr.tile([128, 320], f32, name="pr", bufs=2)
                        for cc in range(4):
                            c = g * 4 + cc
                            lhs = hln2[:, c * 128:(c + 1) * 128]
                            nc.tensor.matmul(pl[:, cc * 80:(cc + 1) * 80], lhs,
                                             wsb[f"w1a{l}"][:],
                                             start=True, stop=True)
                            nc.tensor.matmul(pr[:, cc * 80:(cc + 1) * 80], lhs,
                                             wsb[f"w1b{l}"][:],
                                             start=True, stop=True)
                        plv = pl[:].rearrange("p (t b c) -> p b t c",
                                              t=4, b=16, c=5)
                        prv = pr[:].rearrange("p (t b c) -> p b t c",
                                              t=4, b=16, c=5)
                        nc.scalar.copy(l4[:, :, g * 4:(g + 1) * 4, :], plv)
                        nc.vector.tensor_copy(r4[:, :, g * 4:(g + 1) * 4, :],
                                              prv)

                # geometric product, lambda-grouped: for each distinct
                # walsh plane build r~ = r * walsh once (rotating scratch),
                # then each l-blade term is one multiply against an
                # XOR-shuffled view plus one accumulate, all on the DVE
                with tc.tile_pool(name=f"pgp_{l}_{_rep}", bufs=1,
                                  space="PSUM") as pgp, \
                     tc.tile_pool(name=f"strm_{l}_{_rep}", bufs=2) as strm:
                    gpE = pgp.tile([128, 640], f32, name="gpE")
                    gpO = pgp.tile([128, 640], f32, name="gpO")

                    def emit_mult(out_t, rt, i, out_off, r_off, sz, x,
                                  accum):
                        del out_t
                        nb = sz.bit_length() - 1
                        runs = []
                        for b in range(nb - 1, -1, -1):
                            f = (x >> b) & 1
                            if runs and runs[-1][1] == f:
                                runs[-1][0] *= 2
                            else:
                                runs.append([2, f])
                        if len(runs) > 2:
                            h2 = sz // 2
                            top = (x >> (nb - 1)) & 1
                            emit_mult(None, rt, i, out_off,
                                      r_off ^ (top * h2), h2, x & (h2 - 1),
                                      accum)
                            emit_mult(None, rt, i, out_off + h2,
                                      (r_off + h2) ^ (top * h2), h2,
                                      x & (h2 - 1), accum)
                            return
                        sizes = [r[0] for r in runs]
                        names = [f"g{j}" for j in range(len(sizes))]
                        pat = (f"p ({' '.join(names)} c) -> "
                               f"p {' '.join(names)} c")
                        kw = dict(zip(names, sizes))

                        def posview(t, off):
                            return t[:, off * 80:(off + sz) * 80].rearrange(
                                pat, c=80, **kw)

                        rv = posview(rt[:], r_off)
                        for j, (_, f) in enumerate(runs):
                            if f:
                                idx = [slice(None)] * (2 + len(runs))
                                idx[1 + j] = slice(None, None, -1)
                                rv = rv[tuple(idx)]
                        lv = ltok[:, SPOS[i] * 80:SPOS[i] * 80 + 80]
                        for _ in range(len(runs)):
                            lv = lv.unsqueeze(1)
                        lv = lv.broadcast_to([128] + sizes + [80])
                        tv = posview(accum[:], out_off)
                        nc.vector.tensor_tensor(tv, lv, rv, ALU.mult)

                    groups = {}
                    for i in range(16):
                        groups.setdefault(LAM[i], []).append(i)
                    order = [0] + [lam for lam in groups if lam != 0]
                    evens = [i for i in range(16) if i % 2 == 0]
                    odds = [i for i in range(16) if i % 2]
                    last_e, last_o = evens[-1], odds[-1]
                    seen_e, seen_o = [], []
                    for lam in order:
                        if lam == 0:
                            rt = rtok
                        else:
                            vi = NLAM.index(lam)
                            rt = strm.tile([128, 1280], f32, name="rvar",
                                           bufs=2)
                            wv_ = (wsb["walsh"][:, vi * 16:(vi + 1) * 16]
                                   .rearrange("p (a b) -> p a b", a=2, b=8)
                                   .unsqueeze(3).broadcast_to([128, 2, 8, 80]))
                            nc.vector.tensor_tensor(
                                rt[:].rearrange("p (a b c) -> p a b c",
                                                a=2, b=8, c=80),
                                rtok[:].rearrange("p (a b c) -> p a b c",
                                                  a=2, b=8, c=80),
                                wv_, ALU.mult)
                        for i in groups[lam]:
                            oddi = i & 1
                            trm = strm.tile([128, 1280], f32, name="trm",
                                            bufs=2)
                            if oddi:
                                emit_mult(trm, rt, i, 8, 0, 8, i >> 1, trm)
                            else:
                                emit_mult(trm, rt, i, 0, 0, 16, i >> 1, trm)
                            seen_o.append(i)
                            st_o = len(seen_o) == 1
                            if not oddi:
                                seen_e.append(i)
                                st_e = len(seen_e) == 1
                                nc.tensor.matmul(gpE[:, 0:512], idn[:],
                                                 trm[:, 0:512], start=st_e,
                                                 stop=(len(seen_e) == 8))
                                nc.tensor.matmul(gpE[:, 512:640], idn[:],
                                                 trm[:, 512:640], start=st_e,
                                                 stop=(len(seen_e) == 8))
                            nc.tensor.matmul(gpO[:, 0:512], idn[:],
                                             trm[:, 640:1152], start=st_o,
                                             stop=(len(seen_o) == 16))
                            nc.tensor.matmul(gpO[:, 512:640], idn[:],
                                             trm[:, 1152:1280], start=st_o,
                                             stop=(len(seen_o) == 16))

                    # gated gelu on the scalar blade (s-pos 0 = blade 0)
                    gate = sb.tile([128, 80], f32, name=f"gate{l}",
                                   tag="gate", bufs=1)
                    nc.scalar.activation(gate[:], gpE[:, 0:80],
                                         AF.Gelu_apprx_tanh)
                    z = sb.tile([128, 1280], f32, name=f"z{l}", tag="z",
                                bufs=1)
                    gb = gate[:].unsqueeze(1).broadcast_to([128, 8, 80])
                    nc.vector.tensor_tensor(
                        z[:, 0:640].rearrange("p (b c) -> p b c", b=8, c=80),
                        gpE[:].rearrange("p (b c) -> p b c", b=8, c=80),
                        gb, ALU.mult)
                    nc.vector.tensor_tensor(
                        z[:, 640:1280].rearrange("p (b c) -> p b c",
                                                 b=8, c=80),
                        gpO[:].rearrange("p (b c) -> p b c", b=8, c=80),
                        gb, ALU.mult)

                # transpose z -> zT [80, 2048] fp16, then w2 + residual
                h_new = sb.tile([NF, NOWN], f32, name=f"hnew{l}", tag="h_own",
                                bufs=1)
                zT = sb.tile([NF, NOWN], f32, name=f"zT{l}", tag="zT", bufs=1)
                z4 = z[:].rearrange("p (b t c) -> p t b c", b=16, t=16, c=5)
                with tc.tile_pool(name=f"pzt_{l}_{_rep}", bufs=2,
                                  space="PSUM") as pzt, \
                     tc.tile_pool(name=f"szt_{l}_{_rep}", bufs=2) as szt:
                    for g in range(4):
                        # repack 4 chunks to (chunk, pos, ch) contiguous so the
                        # transpose's moving operand is 1-D per partition
                        zc = szt.tile([128, 320], f32, name="zc", bufs=2)
                        nc.vector.tensor_copy(
                            zc[:].rearrange("p (t b c) -> p t b c",
                                            t=4, b=16, c=5),
                            z4[:, g * 4:(g + 1) * 4, :, :])
                        pz = pzt.tile([NF, 512], f32, name="pz", bufs=2)
                        for cc in range(4):
                            nc.tensor.transpose(
                                pz[:, cc * 128:(cc + 1) * 128],
                                zc[:, cc * 80:(cc + 1) * 80], idn[:])
                        nc.vector.tensor_copy(zT[:, g * 512:(g + 1) * 512],
                                              pz[:])
                with tc.tile_pool(name=f"pw2_{l}_{_rep}", bufs=2,
                                  space="PSUM") as pw2:
                    for s in range(NOWN // 512):
                        sl = slice(s * 512, (s + 1) * 512)
                        pm = pw2.tile([NF, 512], f32, name="pm", bufs=2)
                        nc.tensor.matmul(pm[:], wsb[f"w2{l}"][:], zT[:, sl],
                                         start=True, stop=True)
                        nc.vector.tensor_tensor(h_new[:, sl], pm[:],
                                                h_att[:, sl], ALU.add)

                _tap(nc, tc, sb, f"hatt{l}", h_att[:], [NF, NOWN], f32)
                _tap(nc, tc, sb, f"gate{l}", gate[:], [128, 80], f32)
                _tap(nc, tc, sb, f"ltok{l}", ltok[:], [128, 1280], f32)
                _tap(nc, tc, sb, f"rtok{l}", rtok[:], [128, 1280], f32)
                _tap(nc, tc, sb, f"z{l}", z[:], [128, 1280], f32)
                _tap(nc, tc, sb, f"zT{l}", zT[:], [NF, NOWN], f32)
                _tap(nc, tc, sb, f"h{l+1}", h_new[:], [NF, NOWN], f32)
                if l < NBLK - 1:
                    h_r = exchange(h_new, l + 1)
                    _tap(nc, tc, sb, f"hr{l+1}", h_r[:], [NF, NOWN], f16)
                h_own = h_new

            # ------------- output projection (own half, token-major out)
            outT = sb.tile([35, NOWN], f32, name="outT", bufs=1)
            with tc.tile_pool(name=f"pout_{_rep}", bufs=2, space="PSUM") as pout:
                for s in range(NOWN // 512):
                    sl = slice(s * 512, (s + 1) * 512)
                    po = pout.tile([35, 512], f32, name="po", bufs=2)
                    nc.tensor.matmul(po[:], wsb["wsel"][:], h_own[:, sl],
                                     start=True, stop=True)
                    nc.vector.tensor_copy(outT[:, sl], po[:])
                for c in range(NOWN // 128):
                    pot = pout.tile([128, 35], f32, name="pot", bufs=2)
                    nc.tensor.transpose(pot[:], outT[:, c * 128:(c + 1) * 128],
                                        idn[:35, :35])
                    osb = sb.tile([128, 35], f32, name="osb", bufs=2)
                    nc.vector.tensor_copy(osb[:], pot[:])
                    nc.sync.dma_start(out_p[c * 128:(c + 1) * 128, :], osb[:])

            dcc_cm.__exit__(None, None, None)
            sb_cm.__exit__(None, None, None)
        cst_cm.__exit__(None, None, None)

    nc.compile()
    _split_excess_waits(nc)
    return nc


def _get_built(reps=1):
    if reps not in _BUILT:
        _BUILT[reps] = _build_nc(reps)
    return _BUILT[reps]


# ---------------------------------------------------------------- entry point
def _make_in_maps(inputs):
    x = np.asarray(inputs["x"], np.float32)
    consts = _build_consts(*[
        np.asarray(inputs[k], np.float32)
        for k in ("w_in", "w_out", "wq", "wk", "wv", "wo", "w_mlp1", "w_mlp2")
    ])
    in_maps = []
    for i in range(8):
        m = dict(consts)
        half = i % 2
        m["x"] = np.ascontiguousarray(
            x[i // 2, half * NOWN:(half + 1) * NOWN, :])
        in_maps.append(m)
    return in_maps


def _assemble_out(results):
    out = np.zeros((B, N, 35), np.float32)
    for i in range(8):
        half = i % 2
        out[i // 2, half * NOWN:(half + 1) * NOWN, :] = results[i]["out"]
    return out


_RUNNER = None


def _get_runner(nc):
    """Cached jitted SPMD executor (same execution path run_bass_kernel_spmd
    takes under axon, minus the per-call retrace)."""
    global _RUNNER
    if _RUNNER is not None:
        return _RUNNER
    import jax
    from jax.sharding import Mesh, PartitionSpec
    from jax.experimental.shard_map import shard_map
    import concourse.bass2jax as b2j
    import concourse.mybir as mybir

    b2j.install_neuronx_cc_hook()
    partition_name = nc.partition_id_tensor.name if nc.partition_id_tensor else None
    in_names, out_names, out_avals = [], [], []
    for alloc in nc.m.functions[0].allocations:
        if not isinstance(alloc, mybir.MemoryLocationSet):
            continue
        name = alloc.memorylocations[0].name
        if alloc.kind == "ExternalInput":
            if name != partition_name:
                in_names.append(name)
        elif alloc.kind == "ExternalOutput":
            out_names.append(name)
            out_avals.append(jax.core.ShapedArray(
                tuple(alloc.tensor_shape), mybir.dt.np(alloc.dtype)))
    n_params, n_outs = len(in_names), len(out_names)
    all_in = list(in_names) + list(out_names)
    if partition_name is not None:
        all_in.append(partition_name)

    def _body(*args):
        operands = list(args)
        if partition_name is not None:
            operands.append(b2j.partition_id_tensor())
        outs = b2j._bass_exec_p.bind(
            *operands,
            out_avals=tuple(out_avals), in_names=tuple(all_in),
            out_names=tuple(out_names), lowering_input_output_aliases=(),
            sim_require_finite=True, sim_require_nnan=True, nc=nc)
        return tuple(outs)

    devices = jax.devices()[:8]
    mesh = Mesh(np.asarray(devices), ("core",))
    sharded = jax.jit(
        shard_map(_body, mesh=mesh,
                  in_specs=(PartitionSpec("core"),) * (n_params + n_outs),
                  out_specs=(PartitionSpec("core"),) * n_outs,
                  check_rep=False),
        keep_unused=True)
    _RUNNER = (sharded, in_names, out_names, out_avals)
    return _RUNNER


def kernel(x, w_in, w_out, wq, wk, wv, wo, w_mlp1, w_mlp2):
    import jax

    in_maps = _make_in_maps(dict(
        x=x, w_in=w_in, w_out=w_out, wq=wq, wk=wk, wv=wv, wo=wo,
        w_mlp1=w_mlp1, w_mlp2=w_mlp2))
    nc = _get_built()
    sharded, in_names, out_names, out_avals = _get_runner(nc)
    concat_in = [
        np.concatenate([in_maps[c][nm] for c in range(8)], axis=0)
        for nm in in_names
    ]
    concat_zeros = [np.zeros((8 * a.shape[0], *a.shape[1:]), a.dtype)
                    for a in out_avals]
    # retry guard: the first execution after a device-state change has
    # occasionally produced NaNs through the axon tunnel; rerun if non-finite
    for _attempt in range(3):
        outs = sharded(*concat_in, *concat_zeros)
        jax.block_until_ready(outs)
        results = [
            {nm: np.asarray(outs[i]).reshape(8, *out_avals[i].shape)[c]
             for i, nm in enumerate(out_names)}
            for c in range(8)
        ]
        out = _assemble_out(results)
        if np.isfinite(out).all():
            return out
    return out


# revision 35
# speedup vs baseline: 1.1615x; 1.1615x over previous
"""GATr model Bass kernel for 8 TRN2 NeuronCores.

Sharding: core i handles batch i//2, sequence half i%2 (2048 of 4096 tokens),
with x pre-sliced per core on the host. Per layer: LN + QKV run on the own
half; the pair's other half arrives via an AllGather of the residual state
issued right after the previous layer's write and consumed late, so it hides
under the own-half attention chunks; K/V for the remote half are recomputed
locally from it. Attention is flash-style over 32 kv chunks (own chunks
first) with a no-max-subtraction softmax and a ones-column in V producing
denominators in the same PSUM accumulation.

Precision: this network amplifies intermediate noise ~1000x into the output
(the geometric-product gating squares values with ~50x-rms outliers, and the
output tolerance sits only ~5x above the fp32 reassociation floor), so every
tensor stays fp32. The S logits matmul alone runs as the f32r hi/lo 3-term
reconstruction (kh*qh + kh*ql + kl*qh at 1 cyc/row each), which is
hardware-fp32-faithful at 3/4 the fp32 matmul cost.

Geometric product: blade-major s-layout [token, blade-pos(16), (chunk,ch)(80)]
with even blades in positions 0-7 and odd in 8-15 (w1 columns / w2 rows are
host-permuted to make this free). On its support the sign table factors
exactly as S[i,k] = (-1)^<lam(i), k XOR i> with unit effective sign, so terms
sharing a walsh plane reuse one r~ = r * walsh tile (stride-0 broadcast of a
[128,112] constant); each l-blade term is then a single DVE multiply against
an XOR-shuffled view of r~ (bit-runs become grouped dims, flipped runs become
reversals), and the 16 term tiles are accumulated exactly in PSUM by identity
matmuls on the otherwise-idle tensor engine. Odd l-blades only touch odd out
blades, halving their work.
"""

import numpy as np

INV, VM, H, NBLK = 32, 1, 5, 3
B, N = 4, 4096
NOWN = N // 2          # tokens per core (own sequence half)
NF = H * 16            # 80 features
METRIC = (0.0, 1.0, 1.0, 1.0)
T_RAW = (3, 5, 9)      # e01, e02, e03 raw blade masks
GRADE_RAW = [bin(m).count("1") for m in range(16)]
SPOS = [b // 2 if b % 2 == 0 else 8 + b // 2 for b in range(16)]  # s-layout
ISPOS = [0] * 16
for _b in range(16):
    ISPOS[SPOS[_b]] = _b
# permutation for the s-layout feature order: new (pos, ch) <- old (ch, blade)
PERM80 = np.array([[ch * 16 + ISPOS[p] for ch in range(5)] for p in range(16)]
                  ).reshape(-1)


# ---------------------------------------------------------------- host math
def _blade_mul(a, b):
    swaps = sum(bin(a >> (i + 1)).count("1") for i in range(4) if (b >> i) & 1)
    coef = -1.0 if swaps % 2 else 1.0
    for i in range(4):
        if ((a & b) >> i) & 1:
            coef *= METRIC[i]
    return a ^ b, coef


def _sign_table():
    S = np.zeros((16, 16), np.float32)  # S[i,k] = sign of l-blade i -> out k
    for i in range(16):
        for k in range(16):
            m, c = _blade_mul(i, k ^ i)
            S[i, k] = c if m == k else 0.0
    return S


def _walsh_lambdas():
    S = _sign_table()
    lams, effs = [], []
    for i in range(16):
        ks = [k for k in range(16) if S[i, k] != 0]
        hit = None
        for lam in range(16):
            for eps in (1.0, -1.0):
                if all(S[i, k] == eps * (-1) ** bin(lam & k).count("1")
                       for k in ks):
                    hit = (lam, eps)
                    break
            if hit:
                break
        assert hit is not None
        lam, eps = hit
        eff = eps * (-1) ** bin(lam & i).count("1")
        assert eff == 1.0, (i, eff)
        lams.append(lam)
    return lams


LAM = _walsh_lambdas()
NLAM = sorted(set(l for l in LAM if l != 0))  # 7 distinct nonzero lambdas


def _expand_equi(w, n_in, n_out):
    W = np.zeros((n_in * 16, n_out * 16), dtype=np.float32)
    for c in range(16):
        W[c::16, c::16] = w[GRADE_RAW[c]].T
    return W


def _build_consts(w_in, w_out, wq, wk, wv, wo, w_mlp1, w_mlp2):
    consts = {}
    inner = np.array([0.0 if (m & 1) else 1.0 for m in range(16)], np.float32)
    mask80 = np.tile(inner, H)
    scale = 1.0 / np.sqrt(16.0 * H)

    wemb = np.zeros((36, NF), np.float32)
    for o in range(H):
        for i in range(1, INV):
            wemb[i, o * 16] = w_in[0, o, i]
        wemb[35, o * 16] = w_in[0, o, 0]
        for d, c in enumerate(T_RAW):
            wemb[32 + d, o * 16 + c] = w_in[2, o, 0]
    consts["wemb"] = np.ascontiguousarray(wemb[:35])
    consts["bemb"] = np.ascontiguousarray(wemb[35:36].T)  # [80, 1] bias

    wsel = np.zeros((NF, 35), np.float32)
    for j in range(INV):
        for i in range(H):
            wsel[i * 16, j] = w_out[0, j, i]
    for d, c in enumerate(T_RAW):
        for i in range(H):
            wsel[i * 16 + c, 32 + d] = w_out[2, 0, i]
    consts["wsel"] = wsel

    for l in range(NBLK):
        consts[f"wq{l}"] = _expand_equi(wq[l], H, H)
        consts[f"wk{l}"] = (_expand_equi(wk[l], H, H)
                            * (mask80[None, :] * scale))
        consts[f"wv{l}"] = _expand_equi(wv[l], H, H)
        consts[f"wo{l}"] = _expand_equi(wo[l], H, H)
        w1 = _expand_equi(w_mlp1[l], H, 2 * H)   # [80, 160]
        consts[f"w1a{l}"] = np.ascontiguousarray(w1[:, :NF][:, PERM80])
        consts[f"w1b{l}"] = np.ascontiguousarray(w1[:, NF:][:, PERM80])
        consts[f"w2{l}"] = np.ascontiguousarray(
            _expand_equi(w_mlp2[l], H, H)[PERM80, :])

    # walsh sign planes over s-layout positions, one 16-wide row per
    # nonzero lambda; broadcast over ck at use via a stride-0 last dim
    wl = np.zeros((len(NLAM), 16), np.float32)
    for vi, lam in enumerate(NLAM):
        for p in range(16):
            wl[vi, p] = (-1.0) ** bin(lam & ISPOS[p]).count("1")
    consts["walsh"] = np.ascontiguousarray(
        np.broadcast_to(wl.reshape(1, -1), (128, len(NLAM) * 16)))
    consts["lnmask"] = np.ascontiguousarray(mask80.reshape(NF, 1))
    return consts


# ---------------------------------------------------------------- bass build
_BUILT = {}
TAPS = []  # debug: intermediate tile names to dump as extra outputs


def _tap(nc, tc, sb, name, tile, shape, dtype):
    if name not in TAPS:
        return
    import concourse.mybir as mybir
    p = nc.declare_dram_parameter(f"tap_{name}", shape, dtype, isOutput=True)
    nc.sync.dma_start(p[:], tile)


def _split_excess_waits(nc, max_waits=1):
    """walrus setupSyncWait rejects >1 sem wait on one instruction; move
    excess waits onto same-engine carrier nops placed just before."""
    import bass_rust

    for bb in nc.main_func.blocks:
        il = bb.instructions
        i = 0
        while i < len(il):
            ins = il[i]
            si = ins.sync_info
            if si is not None and si.on_wait and len(si.on_wait) > max_waits:
                waits = list(si.on_wait)
                si.on_wait = waits[:max_waits]
                excess = waits[max_waits:]
                carriers = []
                for j in range(0, len(excess), max_waits):
                    nop = nc.engines[ins.engine].nop(nofuse=True)
                    nop.ins.sync_info = bass_rust.SyncInfo(
                        on_wait=excess[j : j + max_waits], on_update=[]
                    )
                    carriers.append(nop.ins)
                for bb2 in nc.main_func.blocks:
                    il2 = bb2.instructions
                    for c in carriers:
                        for k in range(len(il2) - 1, -1, -1):
                            if il2[k].name == c.name:
                                il2.pop(k)
                i = next(k for k, x in enumerate(il) if x.name == ins.name)
                for c in reversed(carriers):
                    il.insert(i, c)
                i += len(carriers)
            i += 1


def _build_nc(reps=1):
    import concourse.bass as bass
    import concourse.mybir as mybir
    import concourse.tile as tile
    from concourse import bacc
    from concourse.bass import ds
    from concourse.masks import make_identity

    f32 = mybir.dt.float32
    f32r = mybir.dt.float32r
    f16 = mybir.dt.float16
    bf16 = mybir.dt.bfloat16
    AF = mybir.ActivationFunctionType
    ALU = mybir.AluOpType

    nc = bacc.Bacc("TRN2", target_bir_lowering=False, debug=False, num_devices=8)

    x_in = nc.declare_dram_parameter("x", [NOWN, 35], f32, isOutput=False)
    out_p = nc.declare_dram_parameter("out", [NOWN, 35], f32, isOutput=True)
    shapes = {"wemb": [35, NF], "bemb": [NF, 1], "wsel": [NF, 35],
              "walsh": [128, len(NLAM) * 16], "lnmask": [NF, 1]}
    dtypes = {"wemb": f32, "bemb": f32, "wsel": f32, "walsh": f32,
              "lnmask": f32}
    wnames = ["wemb", "bemb", "wsel", "walsh", "lnmask"]
    for l in range(NBLK):
        for nm in ("wq", "wk", "wv", "wo", "w1a", "w1b", "w2"):
            wnames.append(f"{nm}{l}")
            shapes[f"{nm}{l}"] = [NF, NF]
            dtypes[f"{nm}{l}"] = f32
    wp = {
        name: nc.declare_dram_parameter(name, shapes[name], dtypes[name],
                                        isOutput=False)
        for name in wnames
    }

    with tile.TileContext(nc) as tc:
        pid = nc.partition_id()
        roff = ((pid + 1) % 2) * NF   # remote half rows in the AllGather out

        cst_cm = tc.tile_pool(name="cst", bufs=1)
        cst = cst_cm.__enter__()
        idn = cst.tile([128, 128], f32, name="idn")
        make_identity(nc, idn[:])
        ones_f = cst.tile([1, NF], f32, name="ones_f")
        nc.vector.memset(ones_f[:], 1.0)
        eps_t = cst.tile([1, 1], f32, name="eps_t")
        nc.vector.memset(eps_t[:], 1e-6)
        wsb = {}
        for name in wnames:
            t = cst.tile(shapes[name], dtypes[name], name=f"sb_{name}")
            nc.sync.dma_start(t[:], wp[name][:])
            wsb[name] = t

        for _rep in range(reps):
            sb_cm = tc.tile_pool(name=f"sb_{_rep}", bufs=1)
            sb = sb_cm.__enter__()

            # ------------- embedding: own x -> h_own [80, 2048] fp32
            h_own = sb.tile([NF, NOWN], f32, name="h_emb", tag="h_own", bufs=1)
            with tc.tile_pool(name=f"pemb_{_rep}", bufs=2, space="PSUM") as pemb, \
                 tc.tile_pool(name=f"sbemb_{_rep}", bufs=3) as sbemb:
                for s in range(NOWN // 512):
                    xa = sbemb.tile([128, 4 * 35], f32, name="xa", bufs=2)
                    xav = xa[:].rearrange("p (j c) -> p j c", j=4, c=35)
                    nc.sync.dma_start(
                        xav, x_in[:].rearrange("(s j p) c -> s p j c",
                                               s=4, j=4, p=128)[s])
                    xT = sbemb.tile([35, 512], f32, name="xT", bufs=2)
                    for j in range(4):
                        pxt = pemb.tile([35, 128], f32, name="pxt", bufs=2)
                        nc.tensor.transpose(pxt[:], xav[:, j, :], idn[:])
                        nc.vector.tensor_copy(xT[0:35, j * 128:(j + 1) * 128],
                                              pxt[:])
                    pe = pemb.tile([NF, 512], f32, name="pe", bufs=2)
                    nc.tensor.matmul(pe[:], wsb["wemb"][:], xT[:],
                                     start=True, stop=True)
                    nc.vector.tensor_scalar_add(
                        h_own[:, s * 512:(s + 1) * 512], pe[:], wsb["bemb"][:])

            # ------------- pair exchange of the residual state (fp16)
            dcc_cm = tc.tile_pool(name=f"dcc_{_rep}", bufs=1, space="DRAM")
            dcc = dcc_cm.__enter__()

            def exchange(h_src, idx):
                ccin = dcc.tile([NF, NOWN], f32, name=f"ccin{idx}")
                ccout = dcc.tile([2 * NF, NOWN], f32, name=f"ccout{idx}")
                nc.sync.dma_start(ccin[:], h_src[:])
                nc.gpsimd.collective_compute(
                    "AllGather", mybir.AluOpType.bypass,
                    replica_groups=[[0, 1], [2, 3], [4, 5], [6, 7]],
                    ins=[ccin[:]], outs=[ccout[:]])
                h_r = sb.tile([NF, NOWN], f32, name=f"hr{idx}", tag="h_r",
                              bufs=2)
                nc.sync.dma_start(h_r[:], ccout[ds(roff, NF), :])
                return h_r

            h_r = exchange(h_own, 0)
            _tap(nc, tc, sb, "h0", h_own[:], [NF, NOWN], f32)
            _tap(nc, tc, sb, "hr0", h_r[:], [NF, NOWN], f16)

            # ------------- layer norm: src [80, n] -> dst (fp16), chunked
            def layer_norm(src, dst, n, name, pbufs=2):
                rdt = f32
                lhs_m = wsb["lnmask"]
                lhs_o = ones_f
                with tc.tile_pool(name=f"pln_{name}_{_rep}", bufs=1,
                                  space="PSUM") as pln, \
                     tc.tile_pool(name=f"sln_{name}_{_rep}", bufs=2) as sln:
                    for s in range(n // 512):
                        sl = slice(s * 512, (s + 1) * 512)
                        sq = sln.tile([NF, 512], rdt, name="sq", bufs=2)
                        nc.vector.tensor_tensor(sq[:], src[:, sl], src[:, sl],
                                                ALU.mult)
                        pip = pln.tile([1, 512], f32, name="pip",
                                       bufs=pbufs)
                        nc.tensor.matmul(pip[:], lhs_m[:], sq[:],
                                         start=True, stop=True)
                        sd = sln.tile([1, 512], f32, name="sd", bufs=2)
                        nc.scalar.activation(sd[:], pip[:], AF.Sqrt,
                                             bias=eps_t[:], scale=1.0 / float(NF))
                        rs = sln.tile([1, 512], rdt, name="rs", bufs=2)
                        nc.vector.reciprocal(rs[:], sd[:])
                        pbc = pln.tile([NF, 512], f32, name="pbc",
                                       bufs=pbufs)
                        nc.tensor.matmul(pbc[:], lhs_o[:], rs[:],
                                         start=True, stop=True)
                        nc.vector.tensor_tensor(dst[:, sl], src[:, sl], pbc[:],
                                                ALU.mult)

            # ------------- layers
            for l in range(NBLK):
                # LN + QKV: own half fully first (so the PE stream never
                # head-of-line blocks on the in-flight exchange), then remote
                hln = sb.tile([NF, N], f32, name=f"hln{l}", tag="hln", bufs=1)
                _tap(nc, tc, sb, f"hln{l}", hln[:], [NF, N], f32)
                qh = sb.tile([NF, NOWN], f32r, name=f"qh{l}", tag="qh", bufs=1)
                ql = sb.tile([NF, NOWN], f32r, name=f"ql{l}", tag="ql", bufs=1)
                kh = sb.tile([NF, N], f32r, name=f"kh{l}", tag="kh", bufs=1)
                kl = sb.tile([NF, N], f32r, name=f"kl{l}", tag="kl", bufs=1)
                vtok = sb.tile([128, 32 * 81], f32, name=f"v{l}", tag="v",
                               bufs=1)
                nc.vector.memset(
                    vtok[:].rearrange("p (c u) -> p c u", c=32, u=81)[:, :, 80:81],
                    1.0)
                with tc.tile_pool(name=f"pqkv_{l}_{_rep}", bufs=2,
                                  space="PSUM") as pqkv:
                    layer_norm(h_own, hln[:, 0:NOWN], NOWN, f"a{l}")
                    for s in range(NOWN // 512):
                        sl = slice(s * 512, (s + 1) * 512)
                        pq = pqkv.tile([NF, 512], f32, name="pqk", bufs=2)
                        nc.tensor.matmul(pq[:], wsb[f"wq{l}"][:], hln[:, sl],
                                         start=True, stop=True)
                        nc.vector.tensor_copy(qh[:, sl], pq[:])
                        nc.vector.tensor_tensor(ql[:, sl], pq[:],
                                                qh[:, sl].bitcast(f32),
                                                ALU.subtract)
                        pk = pqkv.tile([NF, 512], f32, name="pqk", bufs=2)
                        nc.tensor.matmul(pk[:], wsb[f"wk{l}"][:], hln[:, sl],
                                         start=True, stop=True)
                        nc.vector.tensor_copy(kh[:, sl], pk[:])
                        nc.vector.tensor_tensor(kl[:, sl], pk[:],
                                                kh[:, sl].bitcast(f32),
                                                ALU.subtract)
                    for c in range(16):
                        pv = pqkv.tile([128, NF], f32, name="pv", bufs=2)
                        nc.tensor.matmul(pv[:], hln[:, c * 128:(c + 1) * 128],
                                         wsb[f"wv{l}"][:],
                                         start=True, stop=True)
                        nc.vector.tensor_copy(vtok[:, c * 81:c * 81 + 80],
                                              pv[:])

                # attention: flash accumulation into psAV [81, 2048]; own kv
                # chunks run while the exchange lands, then remote LN/K/V are
                # produced in small psum pools coexisting with the S pipeline
                avsn = sb.tile([NF, NOWN], f32, name=f"avsn{l}", tag="avsn",
                               bufs=1)
                h_att = sb.tile([NF, NOWN], f32, name=f"hatt{l}", tag="h_att",
                                bufs=1)
                with tc.tile_pool(name=f"pAV_{l}_{_rep}", bufs=1,
                                  space="PSUM") as pAV:
                    psAV = pAV.tile([81, NOWN], f32, name="psAV")

                    def s_chunks(c0, c1, wide=False):
                        W = 1024 if wide else 512
                        with tc.tile_pool(name=f"pS_{l}_{c0}_{_rep}", bufs=2,
                                          space="PSUM") as pS, \
                             tc.tile_pool(name=f"sP_{l}_{c0}_{_rep}",
                                          bufs=3) as sP:
                            for c in range(c0, c1):
                                kchh = kh[:, c * 128:(c + 1) * 128]
                                kchl = kl[:, c * 128:(c + 1) * 128]
                                vch = vtok[:, c * 81:c * 81 + 81]
                                for ns in range(NOWN // W):
                                    psS = pS.tile([128, W], f32, name="psS",
                                                  bufs=2)
                                    nj = W // 512
                                    # term-grouped: one lhsT load per term
                                    for ti, (lhsT, rhs) in enumerate(
                                            ((kchh, qh), (kchh, ql),
                                             (kchl, qh))):
                                        for j in range(nj):
                                            qsl = slice(ns * W + j * 512,
                                                        ns * W + (j + 1) * 512)
                                            nc.tensor.matmul(
                                                psS[:, j * 512:(j + 1) * 512],
                                                lhsT, rhs[:, qsl],
                                                start=(ti == 0),
                                                stop=(ti == 2))
                                    pT = sP.tile([128, W], f32, name="pT",
                                                 bufs=3)
                                    nc.scalar.activation(pT[:], psS[:], AF.Exp)
                                    for j in range(nj):
                                        qsl = slice(ns * W + j * 512,
                                                    ns * W + (j + 1) * 512)
                                        nc.tensor.matmul(
                                            psAV[:, qsl], vch,
                                            pT[:, j * 512:(j + 1) * 512],
                                            start=(c == 0), stop=(c == 31))

                    s_chunks(0, 16)
                    # remote half: LN then K/V in their own small pools
                    layer_norm(h_r, hln[:, NOWN:N], NOWN, f"b{l}",
                               pbufs=1)
                    with tc.tile_pool(name=f"pqkr_{l}_{_rep}", bufs=1,
                                      space="PSUM") as pqkr:
                        for s in range(NOWN // 512):
                            sl = slice(NOWN + s * 512, NOWN + (s + 1) * 512)
                            pk = pqkr.tile([NF, 512], f32, name="pk", bufs=1)
                            nc.tensor.matmul(pk[:], wsb[f"wk{l}"][:],
                                             hln[:, sl], start=True, stop=True)
                            nc.scalar.copy(kh[:, sl], pk[:])
                            nc.vector.tensor_tensor(kl[:, sl], pk[:],
                                                    kh[:, sl].bitcast(f32),
                                                    ALU.subtract)
                        for c in range(16, 32):
                            pv = pqkr.tile([128, NF], f32, name="pv", bufs=1)
                            nc.tensor.matmul(pv[:],
                                             hln[:, c * 128:(c + 1) * 128],
                                             wsb[f"wv{l}"][:],
                                             start=True, stop=True)
                            nc.vector.tensor_copy(vtok[:, c * 81:c * 81 + 80],
                                                  pv[:])
                    s_chunks(16, 32, wide=True)

                    # denominators + normalized attention output
                    avs = sb.tile([81, NOWN], f32, name=f"avs{l}", tag="avs",
                                  bufs=1)
                    nc.vector.tensor_copy(avs[:], psAV[:])
                    dnm = sb.tile([1, NOWN], f32, name=f"dnm{l}", tag="dnm",
                                  bufs=1)
                    nc.sync.dma_start(dnm[:], avs[80:81, :])
                    rd = dnm
                    nc.vector.reciprocal(rd[:], dnm[:])
                    with tc.tile_pool(name=f"ppost_{l}_{_rep}", bufs=2,
                                      space="PSUM") as ppost:
                        for s in range(NOWN // 512):
                            sl = slice(s * 512, (s + 1) * 512)
                            pbb = ppost.tile([NF, 512], f32, name="pbb", bufs=2)
                            nc.tensor.matmul(pbb[:], ones_f[:], rd[:, sl],
                                             start=True, stop=True)
                            nc.vector.tensor_tensor(avsn[:, sl], avs[0:NF, sl],
                                                    pbb[:], ALU.mult)
                with tc.tile_pool(name=f"pwo_{l}_{_rep}", bufs=2,
                                  space="PSUM") as pwo:
                    for s in range(NOWN // 512):
                        sl = slice(s * 512, (s + 1) * 512)
                        pw = pwo.tile([NF, 512], f32, name="pw", bufs=2)
                        nc.tensor.matmul(pw[:], wsb[f"wo{l}"][:], avsn[:, sl],
                                         start=True, stop=True)
                        nc.vector.tensor_tensor(h_att[:, sl], pw[:],
                                                h_own[:, sl], ALU.add)

                # ---------------- MLP on own half
                hln2 = sb.tile([NF, NOWN], f32, name=f"hln2{l}", tag="hln2",
                               bufs=1)
                layer_norm(h_att, hln2[:], NOWN, f"m{l}")

                # ltok/rtok layout per partition: pos(16) x chunk(16) x ch(5);
                # w1a/w1b columns are host-permuted to (pos, ch) order, so the
                # per-chunk [128, 80] matmul output scatters into the chunk
                # column of the (pos, chunk, ch) grid.
                ltok = sb.tile([128, 1280], f32, name=f"ltok{l}", tag="ltok",
                               bufs=1)
                rtok = sb.tile([128, 1280], f32, name=f"rtok{l}", tag="rtok",
                               bufs=1)
                l4 = ltok[:].rearrange("p (b t c) -> p b t c", b=16, t=16, c=5)
                r4 = rtok[:].rearrange("p (b t c) -> p b t c", b=16, t=16, c=5)
                with tc.tile_pool(name=f"plr_{l}_{_rep}", bufs=2,
                                  space="PSUM") as plr:
                    for g in range(4):
                        pl = plr.tile([128, 320], f32, name="pl", bufs=2)
                        pr = plr.tile([128, 320], f32, name="pr", bufs=2)
                        for cc in range(4):
                            c = g * 4 + cc
                            lhs = hln2[:, c * 128:(c + 1) * 128]
                            nc.tensor.matmul(pl[:, cc * 80:(cc + 1) * 80], lhs,
                                             wsb[f"w1a{l}"][:],
                                             start=True, stop=True)
                            nc.tensor.matmul(pr[:, cc * 80:(cc + 1) * 80], lhs,
                                             wsb[f"w1b{l}"][:],
                                             start=True, stop=True)
                        plv = pl[:].rearrange("p (t b c) -> p b t c",
                                              t=4, b=16, c=5)
                        prv = pr[:].rearrange("p (t b c) -> p b t c",
                                              t=4, b=16, c=5)
                        nc.scalar.copy(l4[:, :, g * 4:(g + 1) * 4, :], plv)
                        nc.vector.tensor_copy(r4[:, :, g * 4:(g + 1) * 4, :],
                                              prv)

                # geometric product, lambda-grouped: for each distinct
                # walsh plane build r~ = r * walsh once (rotating scratch),
                # then each l-blade term is one multiply against an
                # XOR-shuffled view plus one accumulate, all on the DVE
                with tc.tile_pool(name=f"pgp_{l}_{_rep}", bufs=1,
                                  space="PSUM") as pgp, \
                     tc.tile_pool(name=f"strm_{l}_{_rep}", bufs=2) as strm:
                    gpE = pgp.tile([128, 640], f32, name="gpE")
                    gpO = pgp.tile([128, 640], f32, name="gpO")

                    def emit_mult(out_t, rt, i, out_off, r_off, sz, x,
                                  accum):
                        del out_t
                        nb = sz.bit_length() - 1
                        runs = []
                        for b in range(nb - 1, -1, -1):
                            f = (x >> b) & 1
                            if runs and runs[-1][1] == f:
                                runs[-1][0] *= 2
                            else:
                                runs.append([2, f])
                        if len(runs) > 2:
                            h2 = sz // 2
                            top = (x >> (nb - 1)) & 1
                            emit_mult(None, rt, i, out_off,
                                      r_off ^ (top * h2), h2, x & (h2 - 1),
                                      accum)
                            emit_mult(None, rt, i, out_off + h2,
                                      (r_off + h2) ^ (top * h2), h2,
                                      x & (h2 - 1), accum)
                            return
                        sizes = [r[0] for r in runs]
                        names = [f"g{j}" for j in range(len(sizes))]
                        pat = (f"p ({' '.join(names)} c) -> "
                               f"p {' '.join(names)} c")
                        kw = dict(zip(names, sizes))

                        def posview(t, off):
                            return t[:, off * 80:(off + sz) * 80].rearrange(
                                pat, c=80, **kw)

                        rv = posview(rt[:], r_off)
                        for j, (_, f) in enumerate(runs):
                            if f:
                                idx = [slice(None)] * (2 + len(runs))
                                idx[1 + j] = slice(None, None, -1)
                                rv = rv[tuple(idx)]
                        lv = ltok[:, SPOS[i] * 80:SPOS[i] * 80 + 80]
                        for _ in range(len(runs)):
                            lv = lv.unsqueeze(1)
                        lv = lv.broadcast_to([128] + sizes + [80])
                        tv = posview(accum[:], out_off)
                        nc.vector.tensor_tensor(tv, lv, rv, ALU.mult)

                    groups = {}
                    for i in range(16):
                        groups.setdefault(LAM[i], []).append(i)
                    order = [0] + [lam for lam in groups if lam != 0]
                    evens = [i for i in range(16) if i % 2 == 0]
                    odds = [i for i in range(16) if i % 2]
                    last_e, last_o = evens[-1], odds[-1]
                    seen_e, seen_o = [], []
                    for lam in order:
                        if lam == 0:
                            rt = rtok
                        else:
                            vi = NLAM.index(lam)
                            rt = strm.tile([128, 1280], f32, name="rvar",
                                           bufs=2)
                            wv_ = (wsb["walsh"][:, vi * 16:(vi + 1) * 16]
                                   .rearrange("p (a b) -> p a b", a=2, b=8)
                                   .unsqueeze(3).broadcast_to([128, 2, 8, 80]))
                            nc.vector.tensor_tensor(
                                rt[:].rearrange("p (a b c) -> p a b c",
                                                a=2, b=8, c=80),
                                rtok[:].rearrange("p (a b c) -> p a b c",
                                                  a=2, b=8, c=80),
                                wv_, ALU.mult)
                        for i in groups[lam]:
                            oddi = i & 1
                            trm = strm.tile([128, 1280], f32, name="trm",
                                            bufs=2)
                            if oddi:
                                emit_mult(trm, rt, i, 8, 0, 8, i >> 1, trm)
                            else:
                                emit_mult(trm, rt, i, 0, 0, 16, i >> 1, trm)
                            seen_o.append(i)
                            st_o = len(seen_o) == 1
                            if not oddi:
                                seen_e.append(i)
                                st_e = len(seen_e) == 1
                                nc.tensor.matmul(gpE[:, 0:512], idn[:],
                                                 trm[:, 0:512], start=st_e,
                                                 stop=(len(seen_e) == 8))
                                nc.tensor.matmul(gpE[:, 512:640], idn[:],
                                                 trm[:, 512:640], start=st_e,
                                                 stop=(len(seen_e) == 8))
                            nc.tensor.matmul(gpO[:, 0:512], idn[:],
                                             trm[:, 640:1152], start=st_o,
                                             stop=(len(seen_o) == 16))
                            nc.tensor.matmul(gpO[:, 512:640], idn[:],
                                             trm[:, 1152:1280], start=st_o,
                                             stop=(len(seen_o) == 16))

                    # gated gelu on the scalar blade (s-pos 0 = blade 0)
                    gate = sb.tile([128, 80], f32, name=f"gate{l}",
                                   tag="gate", bufs=1)
                    nc.scalar.activation(gate[:], gpE[:, 0:80],
                                         AF.Gelu_apprx_tanh)
                    z = sb.tile([128, 1280], f32, name=f"z{l}", tag="z",
                                bufs=1)
                    gb = gate[:].unsqueeze(1).broadcast_to([128, 8, 80])
                    nc.vector.tensor_tensor(
                        z[:, 0:640].rearrange("p (b c) -> p b c", b=8, c=80),
                        gpE[:].rearrange("p (b c) -> p b c", b=8, c=80),
                        gb, ALU.mult)
                    nc.vector.tensor_tensor(
                        z[:, 640:1280].rearrange("p (b c) -> p b c",
                                                 b=8, c=80),
                        gpO[:].rearrange("p (b c) -> p b c", b=8, c=80),
                        gb, ALU.mult)

                # transpose z -> zT [80, 2048] fp16, then w2 + residual
                h_new = sb.tile([NF, NOWN], f32, name=f"hnew{l}", tag="h_own",
                                bufs=1)
                zT = sb.tile([NF, NOWN], f32, name=f"zT{l}", tag="zT", bufs=1)
                z4 = z[:].rearrange("p (b t c) -> p t b c", b=16, t=16, c=5)
                with tc.tile_pool(name=f"pzt_{l}_{_rep}", bufs=2,
                                  space="PSUM") as pzt, \
                     tc.tile_pool(name=f"szt_{l}_{_rep}", bufs=2) as szt:
                    for g in range(4):
                        # repack 4 chunks to (chunk, pos, ch) contiguous so the
                        # transpose's moving operand is 1-D per partition
                        zc = szt.tile([128, 320], f32, name="zc", bufs=2)
                        nc.vector.tensor_copy(
                            zc[:].rearrange("p (t b c) -> p t b c",
                                            t=4, b=16, c=5),
                            z4[:, g * 4:(g + 1) * 4, :, :])
                        pz = pzt.tile([NF, 512], f32, name="pz", bufs=2)
                        for cc in range(4):
                            nc.tensor.transpose(
                                pz[:, cc * 128:(cc + 1) * 128],
                                zc[:, cc * 80:(cc + 1) * 80], idn[:])
                        nc.vector.tensor_copy(zT[:, g * 512:(g + 1) * 512],
                                              pz[:])
                with tc.tile_pool(name=f"pw2_{l}_{_rep}", bufs=2,
                                  space="PSUM") as pw2:
                    for s in range(NOWN // 512):
                        sl = slice(s * 512, (s + 1) * 512)
                        pm = pw2.tile([NF, 512], f32, name="pm", bufs=2)
                        nc.tensor.matmul(pm[:], wsb[f"w2{l}"][:], zT[:, sl],
                                         start=True, stop=True)
                        nc.vector.tensor_tensor(h_new[:, sl], pm[:],
                                                h_att[:, sl], ALU.add)

                _tap(nc, tc, sb, f"hatt{l}", h_att[:], [NF, NOWN], f32)
                _tap(nc, tc, sb, f"gate{l}", gate[:], [128, 80], f32)
                _tap(nc, tc, sb, f"ltok{l}", ltok[:], [128, 1280], f32)
                _tap(nc, tc, sb, f"rtok{l}", rtok[:], [128, 1280], f32)
                _tap(nc, tc, sb, f"z{l}", z[:], [128, 1280], f32)
                _tap(nc, tc, sb, f"zT{l}", zT[:], [NF, NOWN], f32)
                _tap(nc, tc, sb, f"h{l+1}", h_new[:], [NF, NOWN], f32)
                if l < NBLK - 1:
                    h_r = exchange(h_new, l + 1)
                    _tap(nc, tc, sb, f"hr{l+1}", h_r[:], [NF, NOWN], f16)
                h_own = h_new

            # ------------- output projection (own half, token-major out)
            outT = sb.tile([35, NOWN], f32, name="outT", bufs=1)
            with tc.tile_pool(name=f"pout_{_rep}", bufs=2, space="PSUM") as pout:
                for s in range(NOWN // 512):
                    sl = slice(s * 512, (s + 1) * 512)
                    po = pout.tile([35, 512], f32, name="po", bufs=2)
                    nc.tensor.matmul(po[:], wsb["wsel"][:], h_own[:, sl],
                                     start=True, stop=True)
                    nc.vector.tensor_copy(outT[:, sl], po[:])
                for c in range(NOWN // 128):
                    pot = pout.tile([128, 35], f32, name="pot", bufs=2)
                    nc.tensor.transpose(pot[:], outT[:, c * 128:(c + 1) * 128],
                                        idn[:35, :35])
                    osb = sb.tile([128, 35], f32, name="osb", bufs=2)
                    nc.vector.tensor_copy(osb[:], pot[:])
                    nc.sync.dma_start(out_p[c * 128:(c + 1) * 128, :], osb[:])

            dcc_cm.__exit__(None, None, None)
            sb_cm.__exit__(None, None, None)
        cst_cm.__exit__(None, None, None)

    nc.compile()
    _split_excess_waits(nc)
    return nc


def _get_built(reps=1):
    if reps not in _BUILT:
        _BUILT[reps] = _build_nc(reps)
    return _BUILT[reps]


# ---------------------------------------------------------------- entry point
def _make_in_maps(inputs):
    x = np.asarray(inputs["x"], np.float32)
    consts = _build_consts(*[
        np.asarray(inputs[k], np.float32)
        for k in ("w_in", "w_out", "wq", "wk", "wv", "wo", "w_mlp1", "w_mlp2")
    ])
    in_maps = []
    for i in range(8):
        m = dict(consts)
        half = i % 2
        m["x"] = np.ascontiguousarray(
            x[i // 2, half * NOWN:(half + 1) * NOWN, :])
        in_maps.append(m)
    return in_maps


def _assemble_out(results):
    out = np.zeros((B, N, 35), np.float32)
    for i in range(8):
        half = i % 2
        out[i // 2, half * NOWN:(half + 1) * NOWN, :] = results[i]["out"]
    return out


_RUNNER = None


def _get_runner(nc):
    """Cached jitted SPMD executor (same execution path run_bass_kernel_spmd
    takes under axon, minus the per-call retrace)."""
    global _RUNNER
    if _RUNNER is not None:
        return _RUNNER
    import jax
    from jax.sharding import Mesh, PartitionSpec
    from jax.experimental.shard_map import shard_map
    import concourse.bass2jax as b2j
    import concourse.mybir as mybir

    b2j.install_neuronx_cc_hook()
    partition_name = nc.partition_id_tensor.name if nc.partition_id_tensor else None
    in_names, out_names, out_avals = [], [], []
    for alloc in nc.m.functions[0].allocations:
        if not isinstance(alloc, mybir.MemoryLocationSet):
            continue
        name = alloc.memorylocations[0].name
        if alloc.kind == "ExternalInput":
            if name != partition_name:
                in_names.append(name)
        elif alloc.kind == "ExternalOutput":
            out_names.append(name)
            out_avals.append(jax.core.ShapedArray(
                tuple(alloc.tensor_shape), mybir.dt.np(alloc.dtype)))
    n_params, n_outs = len(in_names), len(out_names)
    all_in = list(in_names) + list(out_names)
    if partition_name is not None:
        all_in.append(partition_name)

    def _body(*args):
        operands = list(args)
        if partition_name is not None:
            operands.append(b2j.partition_id_tensor())
        outs = b2j._bass_exec_p.bind(
            *operands,
            out_avals=tuple(out_avals), in_names=tuple(all_in),
            out_names=tuple(out_names), lowering_input_output_aliases=(),
            sim_require_finite=True, sim_require_nnan=True, nc=nc)
        return tuple(outs)

    devices = jax.devices()[:8]
    mesh = Mesh(np.asarray(devices), ("core",))
    sharded = jax.jit(
        shard_map(_body, mesh=mesh,
                  in_specs=(PartitionSpec("core"),) * (n_params + n_outs),
                  out_specs=(PartitionSpec("core"),) * n_outs,
                  check_rep=False),
        keep_unused=True)
    _RUNNER = (sharded, in_names, out_names, out_avals)
    return _RUNNER


def kernel(x, w_in, w_out, wq, wk, wv, wo, w_mlp1, w_mlp2):
    import jax

    in_maps = _make_in_maps(dict(
        x=x, w_in=w_in, w_out=w_out, wq=wq, wk=wk, wv=wv, wo=wo,
        w_mlp1=w_mlp1, w_mlp2=w_mlp2))
    nc = _get_built()
    sharded, in_names, out_names, out_avals = _get_runner(nc)
    concat_in = [
        np.concatenate([in_maps[c][nm] for c in range(8)], axis=0)
        for nm in in_names
    ]
    concat_zeros = [np.zeros((8 * a.shape[0], *a.shape[1:]), a.dtype)
                    for a in out_avals]
    # retry guard: the first execution after a device-state change has
    # occasionally produced NaNs through the axon tunnel; rerun if non-finite
    for _attempt in range(3):
        outs = sharded(*concat_in, *concat_zeros)
        jax.block_until_ready(outs)
        results = [
            {nm: np.asarray(outs[i]).reshape(8, *out_avals[i].shape)[c]
             for i, nm in enumerate(out_names)}
            for c in range(8)
        ]
        out = _assemble_out(results)
        if np.isfinite(out).all():
            return out
    return out


# revision 36
# speedup vs baseline: 1.3743x; 1.1832x over previous
"""GATr model Bass kernel for 8 TRN2 NeuronCores.

Sharding: core i handles batch i//2, sequence half i%2 (2048 of 4096 tokens),
with x pre-sliced per core on the host. Per layer: LN + QKV run on the own
half; the pair's other half arrives via an AllGather of the residual state
issued right after the previous layer's write and consumed late, so it hides
under the own-half attention chunks; K/V for the remote half are recomputed
locally from it. Attention is flash-style over 32 kv chunks (own chunks
first) with a no-max-subtraction softmax and a ones-column in V producing
denominators in the same PSUM accumulation.

Precision: this network amplifies intermediate noise ~1000x into the output
(the geometric-product gating squares values with ~50x-rms outliers, and the
output tolerance sits only ~5x above the fp32 reassociation floor), so every
tensor stays fp32. The S logits matmul alone runs as the f32r hi/lo 3-term
reconstruction (kh*qh + kh*ql + kl*qh at 1 cyc/row each), which is
hardware-fp32-faithful at 3/4 the fp32 matmul cost.

Geometric product: blade-major s-layout [token, blade-pos(16), (chunk,ch)(80)]
with even blades in positions 0-7 and odd in 8-15 (w1 columns / w2 rows are
host-permuted to make this free). On its support the sign table factors
exactly as S[i,k] = (-1)^<lam(i), k XOR i> with unit effective sign, so terms
sharing a walsh plane reuse one r~ = r * walsh tile (stride-0 broadcast of a
[128,112] constant); each l-blade term is then a single DVE multiply against
an XOR-shuffled view of r~ (bit-runs become grouped dims, flipped runs become
reversals), and the 16 term tiles are accumulated exactly in PSUM by identity
matmuls on the otherwise-idle tensor engine. Odd l-blades only touch odd out
blades, halving their work.
"""

import numpy as np

INV, VM, H, NBLK = 32, 1, 5, 3
B, N = 4, 4096
NOWN = N // 2          # tokens per core (own sequence half)
NF = H * 16            # 80 features
METRIC = (0.0, 1.0, 1.0, 1.0)
T_RAW = (3, 5, 9)      # e01, e02, e03 raw blade masks
GRADE_RAW = [bin(m).count("1") for m in range(16)]
SPOS = [b // 2 if b % 2 == 0 else 8 + b // 2 for b in range(16)]  # s-layout
ISPOS = [0] * 16
for _b in range(16):
    ISPOS[SPOS[_b]] = _b
# permutation for the s-layout feature order: new (pos, ch) <- old (ch, blade)
PERM80 = np.array([[ch * 16 + ISPOS[p] for ch in range(5)] for p in range(16)]
                  ).reshape(-1)


# ---------------------------------------------------------------- host math
def _blade_mul(a, b):
    swaps = sum(bin(a >> (i + 1)).count("1") for i in range(4) if (b >> i) & 1)
    coef = -1.0 if swaps % 2 else 1.0
    for i in range(4):
        if ((a & b) >> i) & 1:
            coef *= METRIC[i]
    return a ^ b, coef


def _sign_table():
    S = np.zeros((16, 16), np.float32)  # S[i,k] = sign of l-blade i -> out k
    for i in range(16):
        for k in range(16):
            m, c = _blade_mul(i, k ^ i)
            S[i, k] = c if m == k else 0.0
    return S


def _walsh_lambdas():
    S = _sign_table()
    lams, effs = [], []
    for i in range(16):
        ks = [k for k in range(16) if S[i, k] != 0]
        hit = None
        for lam in range(16):
            for eps in (1.0, -1.0):
                if all(S[i, k] == eps * (-1) ** bin(lam & k).count("1")
                       for k in ks):
                    hit = (lam, eps)
                    break
            if hit:
                break
        assert hit is not None
        lam, eps = hit
        eff = eps * (-1) ** bin(lam & i).count("1")
        assert eff == 1.0, (i, eff)
        lams.append(lam)
    return lams


LAM = _walsh_lambdas()
NLAM = sorted(set(l for l in LAM if l != 0))  # 7 distinct nonzero lambdas


def _expand_equi(w, n_in, n_out):
    W = np.zeros((n_in * 16, n_out * 16), dtype=np.float32)
    for c in range(16):
        W[c::16, c::16] = w[GRADE_RAW[c]].T
    return W


def _build_consts(w_in, w_out, wq, wk, wv, wo, w_mlp1, w_mlp2):
    consts = {}
    inner = np.array([0.0 if (m & 1) else 1.0 for m in range(16)], np.float32)
    mask80 = np.tile(inner, H)
    scale = 1.0 / np.sqrt(16.0 * H)

    wemb = np.zeros((36, NF), np.float32)
    for o in range(H):
        for i in range(1, INV):
            wemb[i, o * 16] = w_in[0, o, i]
        wemb[35, o * 16] = w_in[0, o, 0]
        for d, c in enumerate(T_RAW):
            wemb[32 + d, o * 16 + c] = w_in[2, o, 0]
    consts["wemb"] = np.ascontiguousarray(wemb[:35])
    consts["bemb"] = np.ascontiguousarray(wemb[35:36].T)  # [80, 1] bias

    wsel = np.zeros((NF, 35), np.float32)
    for j in range(INV):
        for i in range(H):
            wsel[i * 16, j] = w_out[0, j, i]
    for d, c in enumerate(T_RAW):
        for i in range(H):
            wsel[i * 16 + c, 32 + d] = w_out[2, 0, i]
    consts["wsel"] = wsel

    for l in range(NBLK):
        consts[f"wq{l}"] = _expand_equi(wq[l], H, H)
        consts[f"wk{l}"] = (_expand_equi(wk[l], H, H)
                            * (mask80[None, :] * scale))
        consts[f"wv{l}"] = _expand_equi(wv[l], H, H)
        consts[f"wo{l}"] = _expand_equi(wo[l], H, H)
        w1 = _expand_equi(w_mlp1[l], H, 2 * H)   # [80, 160]
        consts[f"w1a{l}"] = np.ascontiguousarray(w1[:, :NF][:, PERM80])
        consts[f"w1b{l}"] = np.ascontiguousarray(w1[:, NF:][:, PERM80])
        consts[f"w2{l}"] = np.ascontiguousarray(
            _expand_equi(w_mlp2[l], H, H)[PERM80, :])

    # walsh sign planes over s-layout positions, one 16-wide row per
    # nonzero lambda; broadcast over ck at use via a stride-0 last dim
    wl = np.zeros((len(NLAM), 16), np.float32)
    for vi, lam in enumerate(NLAM):
        for p in range(16):
            wl[vi, p] = (-1.0) ** bin(lam & ISPOS[p]).count("1")
    consts["walsh"] = np.ascontiguousarray(
        np.broadcast_to(wl.reshape(1, -1), (128, len(NLAM) * 16)))
    consts["lnmask"] = np.ascontiguousarray(mask80.reshape(NF, 1))
    return consts


# ---------------------------------------------------------------- bass build
_BUILT = {}
TAPS = []  # debug: intermediate tile names to dump as extra outputs


def _tap(nc, tc, sb, name, tile, shape, dtype):
    if name not in TAPS:
        return
    import concourse.mybir as mybir
    p = nc.declare_dram_parameter(f"tap_{name}", shape, dtype, isOutput=True)
    nc.sync.dma_start(p[:], tile)


def _split_excess_waits(nc, max_waits=1):
    """walrus setupSyncWait rejects >1 sem wait on one instruction; move
    excess waits onto same-engine carrier nops placed just before."""
    import bass_rust

    for bb in nc.main_func.blocks:
        il = bb.instructions
        i = 0
        while i < len(il):
            ins = il[i]
            si = ins.sync_info
            if si is not None and si.on_wait and len(si.on_wait) > max_waits:
                waits = list(si.on_wait)
                si.on_wait = waits[:max_waits]
                excess = waits[max_waits:]
                carriers = []
                for j in range(0, len(excess), max_waits):
                    nop = nc.engines[ins.engine].nop(nofuse=True)
                    nop.ins.sync_info = bass_rust.SyncInfo(
                        on_wait=excess[j : j + max_waits], on_update=[]
                    )
                    carriers.append(nop.ins)
                for bb2 in nc.main_func.blocks:
                    il2 = bb2.instructions
                    for c in carriers:
                        for k in range(len(il2) - 1, -1, -1):
                            if il2[k].name == c.name:
                                il2.pop(k)
                i = next(k for k, x in enumerate(il) if x.name == ins.name)
                for c in reversed(carriers):
                    il.insert(i, c)
                i += len(carriers)
            i += 1


def _build_nc(reps=1):
    import concourse.bass as bass
    import concourse.mybir as mybir
    import concourse.tile as tile
    from concourse import bacc
    from concourse.bass import ds
    from concourse.masks import make_identity

    f32 = mybir.dt.float32
    f32r = mybir.dt.float32r
    f16 = mybir.dt.float16
    bf16 = mybir.dt.bfloat16
    AF = mybir.ActivationFunctionType
    ALU = mybir.AluOpType

    nc = bacc.Bacc("TRN2", target_bir_lowering=False, debug=False, num_devices=8)

    x_in = nc.declare_dram_parameter("x", [NOWN, 35], f32, isOutput=False)
    out_p = nc.declare_dram_parameter("out", [NOWN, 35], f32, isOutput=True)
    shapes = {"wemb": [35, NF], "bemb": [NF, 1], "wsel": [NF, 35],
              "walsh": [128, len(NLAM) * 16], "lnmask": [NF, 1]}
    dtypes = {"wemb": f32, "bemb": f32, "wsel": f32, "walsh": f32,
              "lnmask": f32}
    wnames = ["wemb", "bemb", "wsel", "walsh", "lnmask"]
    for l in range(NBLK):
        for nm in ("wq", "wk", "wv", "wo", "w1a", "w1b", "w2"):
            wnames.append(f"{nm}{l}")
            shapes[f"{nm}{l}"] = [NF, NF]
            dtypes[f"{nm}{l}"] = f32
    wp = {
        name: nc.declare_dram_parameter(name, shapes[name], dtypes[name],
                                        isOutput=False)
        for name in wnames
    }

    with tile.TileContext(nc) as tc:
        pid = nc.partition_id()
        roff = ((pid + 1) % 2) * NF   # remote half rows in the AllGather out

        cst_cm = tc.tile_pool(name="cst", bufs=1)
        cst = cst_cm.__enter__()
        idn = cst.tile([128, 128], f32, name="idn")
        make_identity(nc, idn[:])
        ones_f = cst.tile([1, NF], f32, name="ones_f")
        nc.vector.memset(ones_f[:], 1.0)
        eps_t = cst.tile([1, 1], f32, name="eps_t")
        nc.vector.memset(eps_t[:], 1e-6)
        wsb = {}
        for name in wnames:
            t = cst.tile(shapes[name], dtypes[name], name=f"sb_{name}")
            nc.sync.dma_start(t[:], wp[name][:])
            wsb[name] = t

        for _rep in range(reps):
            sb_cm = tc.tile_pool(name=f"sb_{_rep}", bufs=1)
            sb = sb_cm.__enter__()

            # ------------- embedding: own x -> h_own [80, 2048] fp32
            h_own = sb.tile([NF, NOWN], f32, name="h_emb", tag="h_own", bufs=1)
            with tc.tile_pool(name=f"pemb_{_rep}", bufs=2, space="PSUM") as pemb, \
                 tc.tile_pool(name=f"sbemb_{_rep}", bufs=3) as sbemb:
                for s in range(NOWN // 512):
                    xa = sbemb.tile([128, 4 * 35], f32, name="xa", bufs=2)
                    xav = xa[:].rearrange("p (j c) -> p j c", j=4, c=35)
                    nc.sync.dma_start(
                        xav, x_in[:].rearrange("(s j p) c -> s p j c",
                                               s=4, j=4, p=128)[s])
                    xT = sbemb.tile([35, 512], f32, name="xT", bufs=2)
                    for j in range(4):
                        pxt = pemb.tile([35, 128], f32, name="pxt", bufs=2)
                        nc.tensor.transpose(pxt[:], xav[:, j, :], idn[:])
                        nc.vector.tensor_copy(xT[0:35, j * 128:(j + 1) * 128],
                                              pxt[:])
                    pe = pemb.tile([NF, 512], f32, name="pe", bufs=2)
                    nc.tensor.matmul(pe[:], wsb["wemb"][:], xT[:],
                                     start=True, stop=True)
                    nc.vector.tensor_scalar_add(
                        h_own[:, s * 512:(s + 1) * 512], pe[:], wsb["bemb"][:])

            # ------------- pair exchange of the residual state (fp16)
            dcc_cm = tc.tile_pool(name=f"dcc_{_rep}", bufs=1, space="DRAM")
            dcc = dcc_cm.__enter__()

            def exchange(h_src, idx):
                ccin = dcc.tile([NF, NOWN], f32, name=f"ccin{idx}")
                ccout = dcc.tile([2 * NF, NOWN], f32, name=f"ccout{idx}")
                nc.sync.dma_start(ccin[:], h_src[:])
                nc.gpsimd.collective_compute(
                    "AllGather", mybir.AluOpType.bypass,
                    replica_groups=[[0, 1], [2, 3], [4, 5], [6, 7]],
                    ins=[ccin[:]], outs=[ccout[:]])
                h_r = sb.tile([NF, NOWN], f32, name=f"hr{idx}", tag="h_r",
                              bufs=2)
                nc.sync.dma_start(h_r[:], ccout[ds(roff, NF), :])
                return h_r

            h_r = exchange(h_own, 0)
            _tap(nc, tc, sb, "h0", h_own[:], [NF, NOWN], f32)
            _tap(nc, tc, sb, "hr0", h_r[:], [NF, NOWN], f16)

            # ------------- layer norm: src [80, n] -> dst (fp16), chunked
            def layer_norm(src, dst, n, name, pbufs=2):
                rdt = f32
                lhs_m = wsb["lnmask"]
                lhs_o = ones_f
                with tc.tile_pool(name=f"pln_{name}_{_rep}", bufs=1,
                                  space="PSUM") as pln, \
                     tc.tile_pool(name=f"sln_{name}_{_rep}", bufs=2) as sln:
                    for s in range(n // 512):
                        sl = slice(s * 512, (s + 1) * 512)
                        sq = sln.tile([NF, 512], rdt, name="sq", bufs=2)
                        nc.vector.tensor_tensor(sq[:], src[:, sl], src[:, sl],
                                                ALU.mult)
                        pip = pln.tile([1, 512], f32, name="pip",
                                       bufs=pbufs)
                        nc.tensor.matmul(pip[:], lhs_m[:], sq[:],
                                         start=True, stop=True)
                        sd = sln.tile([1, 512], f32, name="sd", bufs=2)
                        nc.scalar.activation(sd[:], pip[:], AF.Sqrt,
                                             bias=eps_t[:], scale=1.0 / float(NF))
                        rs = sln.tile([1, 512], rdt, name="rs", bufs=2)
                        nc.vector.reciprocal(rs[:], sd[:])
                        pbc = pln.tile([NF, 512], f32, name="pbc",
                                       bufs=pbufs)
                        nc.tensor.matmul(pbc[:], lhs_o[:], rs[:],
                                         start=True, stop=True)
                        nc.vector.tensor_tensor(dst[:, sl], src[:, sl], pbc[:],
                                                ALU.mult)

            # ------------- layers
            for l in range(NBLK):
                # LN + QKV: own half fully first (so the PE stream never
                # head-of-line blocks on the in-flight exchange), then remote
                hln = sb.tile([NF, N], f32, name=f"hln{l}", tag="hln", bufs=1)
                _tap(nc, tc, sb, f"hln{l}", hln[:], [NF, N], f32)
                qh = sb.tile([NF, NOWN], f32r, name=f"qh{l}", tag="qh", bufs=1)
                ql = sb.tile([NF, NOWN], f32r, name=f"ql{l}", tag="ql", bufs=1)
                kh = sb.tile([NF, N], f32r, name=f"kh{l}", tag="kh", bufs=1)
                kl = sb.tile([NF, N], f32r, name=f"kl{l}", tag="kl", bufs=1)
                vtok = sb.tile([128, 32 * 81], f32, name=f"v{l}", tag="v",
                               bufs=1)
                nc.vector.memset(
                    vtok[:].rearrange("p (c u) -> p c u", c=32, u=81)[:, :, 80:81],
                    1.0)
                with tc.tile_pool(name=f"pqkv_{l}_{_rep}", bufs=2,
                                  space="PSUM") as pqkv:
                    layer_norm(h_own, hln[:, 0:NOWN], NOWN, f"a{l}")
                    for s in range(NOWN // 512):
                        sl = slice(s * 512, (s + 1) * 512)
                        pq = pqkv.tile([NF, 512], f32, name="pqk", bufs=2)
                        nc.tensor.matmul(pq[:], wsb[f"wq{l}"][:], hln[:, sl],
                                         start=True, stop=True)
                        nc.vector.tensor_copy(qh[:, sl], pq[:])
                        nc.vector.tensor_tensor(ql[:, sl], pq[:],
                                                qh[:, sl].bitcast(f32),
                                                ALU.subtract)
                        pk = pqkv.tile([NF, 512], f32, name="pqk", bufs=2)
                        nc.tensor.matmul(pk[:], wsb[f"wk{l}"][:], hln[:, sl],
                                         start=True, stop=True)
                        nc.vector.tensor_copy(kh[:, sl], pk[:])
                        nc.vector.tensor_tensor(kl[:, sl], pk[:],
                                                kh[:, sl].bitcast(f32),
                                                ALU.subtract)
                    for c in range(16):
                        pv = pqkv.tile([128, NF], f32, name="pv", bufs=2)
                        nc.tensor.matmul(pv[:], hln[:, c * 128:(c + 1) * 128],
                                         wsb[f"wv{l}"][:],
                                         start=True, stop=True)
                        nc.vector.tensor_copy(vtok[:, c * 81:c * 81 + 80],
                                              pv[:])

                # attention: flash accumulation into psAV [81, 2048]; own kv
                # chunks run while the exchange lands, then remote LN/K/V are
                # produced in small psum pools coexisting with the S pipeline
                avsn = sb.tile([NF, NOWN], f32, name=f"avsn{l}", tag="avsn",
                               bufs=1)
                h_att = sb.tile([NF, NOWN], f32, name=f"hatt{l}", tag="h_att",
                                bufs=1)
                with tc.tile_pool(name=f"pAV_{l}_{_rep}", bufs=1,
                                  space="PSUM") as pAV:
                    psAV = pAV.tile([81, NOWN], f32, name="psAV")

                    def s_chunks(c0, c1, wide=False):
                        W = 1024 if wide else 512
                        with tc.tile_pool(name=f"pS_{l}_{c0}_{_rep}", bufs=2,
                                          space="PSUM") as pS, \
                             tc.tile_pool(name=f"sP_{l}_{c0}_{_rep}",
                                          bufs=3) as sP:
                            for c in range(c0, c1):
                                kchh = kh[:, c * 128:(c + 1) * 128]
                                kchl = kl[:, c * 128:(c + 1) * 128]
                                vch = vtok[:, c * 81:c * 81 + 81]
                                for ns in range(NOWN // W):
                                    psS = pS.tile([128, W], f32, name="psS",
                                                  bufs=2)
                                    nj = W // 512
                                    # term-grouped: one lhsT load per term
                                    for ti, (lhsT, rhs) in enumerate(
                                            ((kchh, qh), (kchh, ql),
                                             (kchl, qh))):
                                        for j in range(nj):
                                            qsl = slice(ns * W + j * 512,
                                                        ns * W + (j + 1) * 512)
                                            nc.tensor.matmul(
                                                psS[:, j * 512:(j + 1) * 512],
                                                lhsT, rhs[:, qsl],
                                                start=(ti == 0),
                                                stop=(ti == 2))
                                    pT = sP.tile([128, W], f32, name="pT",
                                                 bufs=3)
                                    nc.scalar.activation(pT[:], psS[:], AF.Exp)
                                    for j in range(nj):
                                        qsl = slice(ns * W + j * 512,
                                                    ns * W + (j + 1) * 512)
                                        nc.tensor.matmul(
                                            psAV[:, qsl], vch,
                                            pT[:, j * 512:(j + 1) * 512],
                                            start=(c == 0), stop=(c == 31))

                    s_chunks(0, 16)
                    # remote half: LN then K/V in their own small pools
                    layer_norm(h_r, hln[:, NOWN:N], NOWN, f"b{l}",
                               pbufs=1)
                    with tc.tile_pool(name=f"pqkr_{l}_{_rep}", bufs=1,
                                      space="PSUM") as pqkr:
                        for s in range(NOWN // 512):
                            sl = slice(NOWN + s * 512, NOWN + (s + 1) * 512)
                            pk = pqkr.tile([NF, 512], f32, name="pk", bufs=1)
                            nc.tensor.matmul(pk[:], wsb[f"wk{l}"][:],
                                             hln[:, sl], start=True, stop=True)
                            nc.scalar.copy(kh[:, sl], pk[:])
                            nc.vector.tensor_tensor(kl[:, sl], pk[:],
                                                    kh[:, sl].bitcast(f32),
                                                    ALU.subtract)
                        for c in range(16, 32):
                            pv = pqkr.tile([128, NF], f32, name="pv", bufs=1)
                            nc.tensor.matmul(pv[:],
                                             hln[:, c * 128:(c + 1) * 128],
                                             wsb[f"wv{l}"][:],
                                             start=True, stop=True)
                            nc.vector.tensor_copy(vtok[:, c * 81:c * 81 + 80],
                                                  pv[:])
                    s_chunks(16, 32, wide=True)

                    # denominators + normalized attention output
                    avs = sb.tile([81, NOWN], f32, name=f"avs{l}", tag="avs",
                                  bufs=1)
                    nc.vector.tensor_copy(avs[:], psAV[:])
                    dnm = sb.tile([1, NOWN], f32, name=f"dnm{l}", tag="dnm",
                                  bufs=1)
                    nc.sync.dma_start(dnm[:], avs[80:81, :])
                    rd = dnm
                    nc.vector.reciprocal(rd[:], dnm[:])
                    with tc.tile_pool(name=f"ppost_{l}_{_rep}", bufs=2,
                                      space="PSUM") as ppost:
                        for s in range(NOWN // 512):
                            sl = slice(s * 512, (s + 1) * 512)
                            pbb = ppost.tile([NF, 512], f32, name="pbb", bufs=2)
                            nc.tensor.matmul(pbb[:], ones_f[:], rd[:, sl],
                                             start=True, stop=True)
                            nc.vector.tensor_tensor(avsn[:, sl], avs[0:NF, sl],
                                                    pbb[:], ALU.mult)
                with tc.tile_pool(name=f"pwo_{l}_{_rep}", bufs=2,
                                  space="PSUM") as pwo:
                    for s in range(NOWN // 512):
                        sl = slice(s * 512, (s + 1) * 512)
                        pw = pwo.tile([NF, 512], f32, name="pw", bufs=2)
                        nc.tensor.matmul(pw[:], wsb[f"wo{l}"][:], avsn[:, sl],
                                         start=True, stop=True)
                        nc.vector.tensor_tensor(h_att[:, sl], pw[:],
                                                h_own[:, sl], ALU.add)

                # ---------------- MLP on own half
                hln2 = sb.tile([NF, NOWN], f32, name=f"hln2{l}", tag="hln2",
                               bufs=1)
                layer_norm(h_att, hln2[:], NOWN, f"m{l}")

                # ltok/rtok layout per partition: pos(16) x chunk(16) x ch(5);
                # w1a/w1b columns are host-permuted to (pos, ch) order, so the
                # per-chunk [128, 80] matmul output scatters into the chunk
                # column of the (pos, chunk, ch) grid.
                ltok = sb.tile([128, 1280], f32, name=f"ltok{l}", tag="ltok",
                               bufs=1)
                rtok = sb.tile([128, 1280], f32, name=f"rtok{l}", tag="rtok",
                               bufs=1)
                l4 = ltok[:].rearrange("p (b t c) -> p b t c", b=16, t=16, c=5)
                r4 = rtok[:].rearrange("p (b t c) -> p b t c", b=16, t=16, c=5)
                with tc.tile_pool(name=f"plr_{l}_{_rep}", bufs=2,
                                  space="PSUM") as plr:
                    for g in range(4):
                        pl = plr.tile([128, 320], f32, name="pl", bufs=2)
                        pr = plr.tile([128, 320], f32, name="pr", bufs=2)
                        for cc in range(4):
                            c = g * 4 + cc
                            lhs = hln2[:, c * 128:(c + 1) * 128]
                            nc.tensor.matmul(pl[:, cc * 80:(cc + 1) * 80], lhs,
                                             wsb[f"w1a{l}"][:],
                                             start=True, stop=True)
                            nc.tensor.matmul(pr[:, cc * 80:(cc + 1) * 80], lhs,
                                             wsb[f"w1b{l}"][:],
                                             start=True, stop=True)
                        plv = pl[:].rearrange("p (t b c) -> p b t c",
                                              t=4, b=16, c=5)
                        prv = pr[:].rearrange("p (t b c) -> p b t c",
                                              t=4, b=16, c=5)
                        nc.scalar.copy(l4[:, :, g * 4:(g + 1) * 4, :], plv)
                        nc.vector.tensor_copy(r4[:, :, g * 4:(g + 1) * 4, :],
                                              prv)

                # geometric product, lambda-grouped: for each distinct
                # walsh plane build r~ = r * walsh once (rotating scratch),
                # then each l-blade term is one multiply against an
                # XOR-shuffled view plus one accumulate, all on the DVE
                with tc.tile_pool(name=f"pgp_{l}_{_rep}", bufs=1,
                                  space="PSUM") as pgp, \
                     tc.tile_pool(name=f"strm_{l}_{_rep}", bufs=2) as strm:
                    gpE = pgp.tile([128, 640], f32, name="gpE")
                    gpO = pgp.tile([128, 640], f32, name="gpO")

                    def emit_mult(out_t, rt, i, out_off, r_off, sz, x,
                                  accum):
                        del out_t
                        nb = sz.bit_length() - 1
                        runs = []
                        for b in range(nb - 1, -1, -1):
                            f = (x >> b) & 1
                            if runs and runs[-1][1] == f:
                                runs[-1][0] *= 2
                            else:
                                runs.append([2, f])
                        if len(runs) > 2:
                            h2 = sz // 2
                            top = (x >> (nb - 1)) & 1
                            emit_mult(None, rt, i, out_off,
                                      r_off ^ (top * h2), h2, x & (h2 - 1),
                                      accum)
                            emit_mult(None, rt, i, out_off + h2,
                                      (r_off + h2) ^ (top * h2), h2,
                                      x & (h2 - 1), accum)
                            return
                        sizes = [r[0] for r in runs]
                        names = [f"g{j}" for j in range(len(sizes))]
                        pat = (f"p ({' '.join(names)} c) -> "
                               f"p {' '.join(names)} c")
                        kw = dict(zip(names, sizes))

                        def posview(t, off):
                            return t[:, off * 80:(off + sz) * 80].rearrange(
                                pat, c=80, **kw)

                        rv = posview(rt[:], r_off)
                        for j, (_, f) in enumerate(runs):
                            if f:
                                idx = [slice(None)] * (2 + len(runs))
                                idx[1 + j] = slice(None, None, -1)
                                rv = rv[tuple(idx)]
                        lv = ltok[:, SPOS[i] * 80:SPOS[i] * 80 + 80]
                        for _ in range(len(runs)):
                            lv = lv.unsqueeze(1)
                        lv = lv.broadcast_to([128] + sizes + [80])
                        tv = posview(accum[:], out_off)
                        nc.vector.tensor_tensor(tv, lv, rv, ALU.mult)

                    groups = {}
                    for i in range(16):
                        groups.setdefault(LAM[i], []).append(i)
                    order = [0] + [lam for lam in groups if lam != 0]
                    evens = [i for i in range(16) if i % 2 == 0]
                    odds = [i for i in range(16) if i % 2]
                    last_e, last_o = evens[-1], odds[-1]
                    seen_e, seen_o = [], []
                    for lam in order:
                        if lam == 0:
                            rt = rtok
                        else:
                            vi = NLAM.index(lam)
                            rt = strm.tile([128, 1280], f32, name="rvar",
                                           bufs=2)  # 2 planes in flight
                            wv_ = (wsb["walsh"][:, vi * 16:(vi + 1) * 16]
                                   .rearrange("p (a b) -> p a b", a=2, b=8)
                                   .unsqueeze(3).broadcast_to([128, 2, 8, 80]))
                            nc.vector.tensor_tensor(
                                rt[:].rearrange("p (a b c) -> p a b c",
                                                a=2, b=8, c=80),
                                rtok[:].rearrange("p (a b c) -> p a b c",
                                                  a=2, b=8, c=80),
                                wv_, ALU.mult)
                        for i in groups[lam]:
                            oddi = i & 1
                            trm = strm.tile([128, 1280], f32, name="trm",
                                            bufs=3)
                            if oddi:
                                emit_mult(trm, rt, i, 8, 0, 8, i >> 1, trm)
                            else:
                                emit_mult(trm, rt, i, 0, 0, 16, i >> 1, trm)
                            seen_o.append(i)
                            st_o = len(seen_o) == 1
                            if not oddi:
                                seen_e.append(i)
                                st_e = len(seen_e) == 1
                                nc.tensor.matmul(gpE[:, 0:512], idn[:],
                                                 trm[:, 0:512], start=st_e,
                                                 stop=(len(seen_e) == 8))
                                nc.tensor.matmul(gpE[:, 512:640], idn[:],
                                                 trm[:, 512:640], start=st_e,
                                                 stop=(len(seen_e) == 8))
                            nc.tensor.matmul(gpO[:, 0:512], idn[:],
                                             trm[:, 640:1152], start=st_o,
                                             stop=(len(seen_o) == 16))
                            nc.tensor.matmul(gpO[:, 512:640], idn[:],
                                             trm[:, 1152:1280], start=st_o,
                                             stop=(len(seen_o) == 16))

                    # gated gelu on the scalar blade (s-pos 0 = blade 0)
                    gate = sb.tile([128, 80], f32, name=f"gate{l}",
                                   tag="gate", bufs=1)
                    nc.scalar.activation(gate[:], gpE[:, 0:80],
                                         AF.Gelu_apprx_tanh)
                    z = sb.tile([128, 1280], f32, name=f"z{l}", tag="z",
                                bufs=1)
                    gb = gate[:].unsqueeze(1).broadcast_to([128, 8, 80])
                    nc.vector.tensor_tensor(
                        z[:, 0:640].rearrange("p (b c) -> p b c", b=8, c=80),
                        gpE[:].rearrange("p (b c) -> p b c", b=8, c=80),
                        gb, ALU.mult)
                    nc.vector.tensor_tensor(
                        z[:, 640:1280].rearrange("p (b c) -> p b c",
                                                 b=8, c=80),
                        gpO[:].rearrange("p (b c) -> p b c", b=8, c=80),
                        gb, ALU.mult)

                # transpose z -> zT [80, 2048] fp16, then w2 + residual
                h_new = sb.tile([NF, NOWN], f32, name=f"hnew{l}", tag="h_own",
                                bufs=1)
                zT = sb.tile([NF, NOWN], f32, name=f"zT{l}", tag="zT", bufs=1)
                z4 = z[:].rearrange("p (b t c) -> p t b c", b=16, t=16, c=5)
                with tc.tile_pool(name=f"pzt_{l}_{_rep}", bufs=2,
                                  space="PSUM") as pzt, \
                     tc.tile_pool(name=f"szt_{l}_{_rep}", bufs=2) as szt:
                    for g in range(4):
                        # repack 4 chunks to (chunk, pos, ch) contiguous so the
                        # transpose's moving operand is 1-D per partition
                        zc = szt.tile([128, 320], f32, name="zc", bufs=2)
                        nc.scalar.copy(
                            zc[:].rearrange("p (t b c) -> p t b c",
                                            t=4, b=16, c=5),
                            z4[:, g * 4:(g + 1) * 4, :, :])
                        pz = pzt.tile([NF, 512], f32, name="pz", bufs=2)
                        for cc in range(4):
                            nc.tensor.transpose(
                                pz[:, cc * 128:(cc + 1) * 128],
                                zc[:, cc * 80:(cc + 1) * 80], idn[:])
                        nc.vector.tensor_copy(zT[:, g * 512:(g + 1) * 512],
                                              pz[:])
                with tc.tile_pool(name=f"pw2_{l}_{_rep}", bufs=2,
                                  space="PSUM") as pw2:
                    for s in range(NOWN // 512):
                        sl = slice(s * 512, (s + 1) * 512)
                        pm = pw2.tile([NF, 512], f32, name="pm", bufs=2)
                        nc.tensor.matmul(pm[:], wsb[f"w2{l}"][:], zT[:, sl],
                                         start=True, stop=True)
                        nc.vector.tensor_tensor(h_new[:, sl], pm[:],
                                                h_att[:, sl], ALU.add)

                _tap(nc, tc, sb, f"hatt{l}", h_att[:], [NF, NOWN], f32)
                _tap(nc, tc, sb, f"gate{l}", gate[:], [128, 80], f32)
                _tap(nc, tc, sb, f"ltok{l}", ltok[:], [128, 1280], f32)
                _tap(nc, tc, sb, f"rtok{l}", rtok[:], [128, 1280], f32)
                _tap(nc, tc, sb, f"z{l}", z[:], [128, 1280], f32)
                _tap(nc, tc, sb, f"zT{l}", zT[:], [NF, NOWN], f32)
                _tap(nc, tc, sb, f"h{l+1}", h_new[:], [NF, NOWN], f32)
                if l < NBLK - 1:
                    h_r = exchange(h_new, l + 1)
                    _tap(nc, tc, sb, f"hr{l+1}", h_r[:], [NF, NOWN], f16)
                h_own = h_new

            # ------------- output projection (own half, token-major out)
            outT = sb.tile([35, NOWN], f32, name="outT", bufs=1)
            with tc.tile_pool(name=f"pout_{_rep}", bufs=2, space="PSUM") as pout:
                for s in range(NOWN // 512):
                    sl = slice(s * 512, (s + 1) * 512)
                    po = pout.tile([35, 512], f32, name="po", bufs=2)
                    nc.tensor.matmul(po[:], wsb["wsel"][:], h_own[:, sl],
                                     start=True, stop=True)
                    nc.vector.tensor_copy(outT[:, sl], po[:])
                for c in range(NOWN // 128):
                    pot = pout.tile([128, 35], f32, name="pot", bufs=2)
                    nc.tensor.transpose(pot[:], outT[:, c * 128:(c + 1) * 128],
                                        idn[:35, :35])
                    osb = sb.tile([128, 35], f32, name="osb", bufs=2)
                    nc.vector.tensor_copy(osb[:], pot[:])
                    nc.sync.dma_start(out_p[c * 128:(c + 1) * 128, :], osb[:])

            dcc_cm.__exit__(None, None, None)
            sb_cm.__exit__(None, None, None)
        cst_cm.__exit__(None, None, None)

    nc.compile()
    _split_excess_waits(nc)
    return nc


def _get_built(reps=1):
    if reps not in _BUILT:
        _BUILT[reps] = _build_nc(reps)
    return _BUILT[reps]


# ---------------------------------------------------------------- entry point
def _make_in_maps(inputs):
    x = np.asarray(inputs["x"], np.float32)
    consts = _build_consts(*[
        np.asarray(inputs[k], np.float32)
        for k in ("w_in", "w_out", "wq", "wk", "wv", "wo", "w_mlp1", "w_mlp2")
    ])
    in_maps = []
    for i in range(8):
        m = dict(consts)
        half = i % 2
        m["x"] = np.ascontiguousarray(
            x[i // 2, half * NOWN:(half + 1) * NOWN, :])
        in_maps.append(m)
    return in_maps


def _assemble_out(results):
    out = np.zeros((B, N, 35), np.float32)
    for i in range(8):
        half = i % 2
        out[i // 2, half * NOWN:(half + 1) * NOWN, :] = results[i]["out"]
    return out


_RUNNER = None


def _get_runner(nc):
    """Cached jitted SPMD executor (same execution path run_bass_kernel_spmd
    takes under axon, minus the per-call retrace)."""
    global _RUNNER
    if _RUNNER is not None:
        return _RUNNER
    import jax
    from jax.sharding import Mesh, PartitionSpec
    from jax.experimental.shard_map import shard_map
    import concourse.bass2jax as b2j
    import concourse.mybir as mybir

    b2j.install_neuronx_cc_hook()
    partition_name = nc.partition_id_tensor.name if nc.partition_id_tensor else None
    in_names, out_names, out_avals = [], [], []
    for alloc in nc.m.functions[0].allocations:
        if not isinstance(alloc, mybir.MemoryLocationSet):
            continue
        name = alloc.memorylocations[0].name
        if alloc.kind == "ExternalInput":
            if name != partition_name:
                in_names.append(name)
        elif alloc.kind == "ExternalOutput":
            out_names.append(name)
            out_avals.append(jax.core.ShapedArray(
                tuple(alloc.tensor_shape), mybir.dt.np(alloc.dtype)))
    n_params, n_outs = len(in_names), len(out_names)
    all_in = list(in_names) + list(out_names)
    if partition_name is not None:
        all_in.append(partition_name)

    def _body(*args):
        operands = list(args)
        if partition_name is not None:
            operands.append(b2j.partition_id_tensor())
        outs = b2j._bass_exec_p.bind(
            *operands,
            out_avals=tuple(out_avals), in_names=tuple(all_in),
            out_names=tuple(out_names), lowering_input_output_aliases=(),
            sim_require_finite=True, sim_require_nnan=True, nc=nc)
        return tuple(outs)

    devices = jax.devices()[:8]
    mesh = Mesh(np.asarray(devices), ("core",))
    sharded = jax.jit(
        shard_map(_body, mesh=mesh,
                  in_specs=(PartitionSpec("core"),) * (n_params + n_outs),
                  out_specs=(PartitionSpec("core"),) * n_outs,
                  check_rep=False),
        keep_unused=True)
    _RUNNER = (sharded, in_names, out_names, out_avals)
    return _RUNNER


def kernel(x, w_in, w_out, wq, wk, wv, wo, w_mlp1, w_mlp2):
    import jax

    in_maps = _make_in_maps(dict(
        x=x, w_in=w_in, w_out=w_out, wq=wq, wk=wk, wv=wv, wo=wo,
        w_mlp1=w_mlp1, w_mlp2=w_mlp2))
    nc = _get_built()
    sharded, in_names, out_names, out_avals = _get_runner(nc)
    concat_in = [
        np.concatenate([in_maps[c][nm] for c in range(8)], axis=0)
        for nm in in_names
    ]
    concat_zeros = [np.zeros((8 * a.shape[0], *a.shape[1:]), a.dtype)
                    for a in out_avals]
    # retry guard: the first execution after a device-state change has
    # occasionally produced NaNs through the axon tunnel; rerun if non-finite
    for _attempt in range(3):
        outs = sharded(*concat_in, *concat_zeros)
        jax.block_until_ready(outs)
        results = [
            {nm: np.asarray(outs[i]).reshape(8, *out_avals[i].shape)[c]
             for i, nm in enumerate(out_names)}
            for c in range(8)
        ]
        out = _assemble_out(results)
        if np.isfinite(out).all():
            return out
    return out
